# revision 16
# baseline (speedup 1.0000x reference)
"""EpisodicMemory Trainium2 kernel (8 NeuronCores, pure data parallel over batch).

Reference semantics (per batch b):
    keys_w   = keys   with row write_ptr[b] <- key[b]
    values_w = values with row write_ptr[b] <- value[b]
    filled_w = min(filled + 1, S)
    query    = hidden @ Wq.T + bq
    scores   = (keys_w @ query) / sqrt(K), masked to s < filled_w
    attn     = softmax(scores)
    retrieved= attn @ values_w
    g        = silu([hidden|retrieved] @ Wg1.T + bg1)
    gate     = sigmoid(g @ Wg2.T + bg2)
    out      = (hidden + gate*retrieved) @ Wo.T + bo

The scatter is never materialized: base scores/retrieved are computed from the
original keys/values and corrected algebraically with the old rows at
write_ptr (host-gathered) plus the new key/value rows.

v3 design:
  * keys host-transposed to [K, S] bf16 -> scores are PE matmuls (contract K).
    16 batches accumulate into one [16, 512] PSUM bank via one-hot query
    columns (out partition offsets must be 0 mod 32, so rows are selected by
    zero-padding the stationary operand instead).
  * values in fp8e4, host-packed in (s%128, s//256, (s//128)%2, v) order for
    perf_mode=DoubleRow matmuls (2 fp8 rows per PE cell); attention weights
    are scaled by 128 into fp8 range (denominator scales identically so the
    softmax normalization cancels the factor).  One-hot diagonal layout lets
    all 16 batches of a group accumulate into one [16, 512] PSUM bank.
  * rows s >= filled_w never contribute (scores masked to -inf), so slot i
    only reads/computes ceil-rounded row counts baked from the host-sorted
    filled profile: batches sorted by filled_w desc, rank 8i+c -> core c
    slot i, so all 8 cores share one compiled program.
"""

import sys

sys.path.insert(0, "/opt/trn_rl_repo")

import numpy as np
import ml_dtypes

import concourse.bacc as bacc
import concourse.tile as tile
from concourse import bass, mybir
from concourse.bass_utils import run_bass_kernel_spmd
from concourse.masks import make_identity

B, S, K, V = 512, 1024, 128, 512
NCORES = 8
NB = B // NCORES          # 64 batches per core
T2 = S // 256             # 4 value double-chunks of 256 rows
GRP = 16                  # batches per softmax group
NG = NB // GRP            # 4 groups
SCALE = float(np.sqrt(K))
NEG_BIG = -3.0e37
LN_ATT = float(np.log(128.0))   # attn weights scaled x128 into fp8 range

F32 = mybir.dt.float32
BF16 = mybir.dt.bfloat16
F8 = mybir.dt.float8e4
NP_BF16 = np.dtype(ml_dtypes.bfloat16)
NP_F8 = np.dtype(ml_dtypes.float8_e4m3)


def _build(pv2, limg):
    """pv2[i]: value 256-row double-chunks for slot i (1..4); limg[g]: key rows
    (multiple of 8) read/scored for group g.  Slots sorted descending."""
    nc = bacc.Bacc()
    dt = F32
    DR = mybir.MatmulPerfMode.DoubleRow

    keysT_t = nc.dram_tensor("keysT", [NB, K, S], F8, kind="ExternalInput")
    vpack_t = nc.dram_tensor("vpack", [NB, 128, T2, 2, V], F8, kind="ExternalInput")
    key_t = nc.dram_tensor("key", [NB, K], dt, kind="ExternalInput")
    value_t = nc.dram_tensor("value", [NB, V], dt, kind="ExternalInput")
    hidden_t = nc.dram_tensor("hidden", [NB, V], dt, kind="ExternalInput")
    filled_t = nc.dram_tensor("filled_f", [NB, 1], dt, kind="ExternalInput")
    wp_t = nc.dram_tensor("wp_f", [NB, 1], dt, kind="ExternalInput")
    kwp_t = nc.dram_tensor("kwp", [NB, K], dt, kind="ExternalInput")
    vwp_t = nc.dram_tensor("vwp", [NB, V], dt, kind="ExternalInput")
    wqT_t = nc.dram_tensor("WqT", [V, K], BF16, kind="ExternalInput")
    wg1T_t = nc.dram_tensor("Wg1T", [2 * V, V], BF16, kind="ExternalInput")
    wg2T_t = nc.dram_tensor("Wg2T", [V, V], BF16, kind="ExternalInput")
    woT_t = nc.dram_tensor("WoT", [V, V], BF16, kind="ExternalInput")
    bq_t = nc.dram_tensor("bq", [K], dt, kind="ExternalInput")
    bg1_t = nc.dram_tensor("bg1", [V], dt, kind="ExternalInput")
    bg2_t = nc.dram_tensor("bg2", [V], dt, kind="ExternalInput")
    bo_t = nc.dram_tensor("bo", [V], dt, kind="ExternalInput")
    out_t = nc.dram_tensor("out", [NB, V], dt, kind="ExternalOutput")

    kview = keysT_t[:].rearrange("b k s -> k b s")

    with tile.TileContext(nc) as tc:
        with (
            tc.tile_pool(name="const", bufs=1) as const,
            tc.tile_pool(name="ktile", bufs=5) as ktile_p,
            tc.tile_pool(name="vtile", bufs=5) as vtile_p,
            tc.tile_pool(name="grp", bufs=2) as grp_p,
            tc.tile_pool(name="sm", bufs=1) as sm_p,
            tc.tile_pool(name="misc", bufs=1) as misc,
            tc.tile_pool(name="ps_sc", bufs=2, space="PSUM") as ps_sc,
            tc.tile_pool(name="ps_gw", bufs=2, space="PSUM") as ps_gw,
            tc.tile_pool(name="ps_tr", bufs=2, space="PSUM") as ps_tr,
        ):
            # ---------------- setup ----------------
            hidden_sb = misc.tile([NB, V], dt)
            nc.scalar.dma_start(out=hidden_sb[:], in_=hidden_t[:, :])
            wqT = const.tile([128, 4, K], BF16)
            nc.scalar.dma_start(out=wqT[:], in_=wqT_t[:].rearrange("(c p) k -> p c k", p=128))
            bq_row = const.tile([1, K], dt)
            nc.scalar.dma_start(out=bq_row[:], in_=bq_t[None, :])

            identity = const.tile([128, 128], dt)
            make_identity(nc, identity[:])
            identity_bf = const.tile([128, 128], BF16)
            nc.vector.tensor_copy(out=identity_bf[:], in_=identity[:])
            ones_row = const.tile([1, 128], dt)
            nc.vector.memset(ones_row[:], 1.0)

            iota_i = misc.tile([GRP, S], mybir.dt.int16)
            nc.gpsimd.iota(iota_i[:], pattern=[[1, S]], base=0, channel_multiplier=0)
            iota_f = const.tile([GRP, S], dt)
            nc.vector.tensor_copy(out=iota_f[:], in_=iota_i[:])

            key_sb = misc.tile([NB, K], dt)
            nc.scalar.dma_start(out=key_sb[:], in_=key_t[:, :])
            value_sb = misc.tile([NB, V], dt)
            nc.scalar.dma_start(out=value_sb[:], in_=value_t[:, :])
            filled_sb = misc.tile([NB, 1], dt)
            nc.scalar.dma_start(out=filled_sb[:], in_=filled_t[:, :])
            wp_sb = misc.tile([NB, 1], dt)
            nc.scalar.dma_start(out=wp_sb[:], in_=wp_t[:, :])
            kwp_sb = misc.tile([NB, K], dt)
            nc.scalar.dma_start(out=kwp_sb[:], in_=kwp_t[:, :])
            vwp_sb = misc.tile([NB, V], dt)
            nc.scalar.dma_start(out=vwp_sb[:], in_=vwp_t[:, :])

            # hidden in bf16 + hiddenT (128v x 64b) chunks for the matmuls
            hidden_bf = misc.tile([NB, V], BF16)
            nc.vector.tensor_copy(out=hidden_bf[:], in_=hidden_sb[:])
            hT = misc.tile([128, 4, NB], BF16)
            for c in range(4):
                tp = ps_tr.tile([128, NB], BF16, tag="tr")
                nc.tensor.transpose(out=tp[:], in_=hidden_bf[:, c * 128:(c + 1) * 128],
                                    identity=identity_bf[:NB, :NB])
                nc.scalar.copy(out=hT[:, c, :], in_=tp[:])

            # query both ways: qT (128k x 64b) for scores, q (64b x 128k) for
            # the write-row correction dot products
            qT_ps = ps_tr.tile([K, NB], dt, tag="tr")
            for c in range(4):
                nc.tensor.matmul(out=qT_ps[:], lhsT=wqT[:, c, :], rhs=hT[:, c, :],
                                 start=(c == 0), stop=False)
            nc.tensor.matmul(out=qT_ps[:], lhsT=bq_row[:], rhs=ones_row[:, :NB],
                             start=False, stop=True)
            qT_f8 = misc.tile([K, NB], F8)
            nc.scalar.copy(out=qT_f8[:], in_=qT_ps[:])

            q_ps = ps_tr.tile([NB, K], dt, tag="tr")
            for c in range(4):
                nc.tensor.matmul(out=q_ps[:], lhsT=hT[:, c, :], rhs=wqT[:, c, :],
                                 start=(c == 0), stop=False)
            nc.tensor.matmul(out=q_ps[:], lhsT=ones_row[:, :NB], rhs=bq_row[:],
                             start=False, stop=True)
            query_sb = misc.tile([NB, K], dt)
            nc.vector.tensor_copy(out=query_sb[:], in_=q_ps[:])

            junk_rd = misc.tile([NB, K], dt)
            sold = misc.tile([NB, 1], dt)
            nc.vector.tensor_mul(out=junk_rd[:], in0=kwp_sb[:], in1=query_sb[:])
            nc.vector.tensor_reduce(out=sold[:], in_=junk_rd[:],
                                    axis=mybir.AxisListType.X, op=mybir.AluOpType.add)
            snew = misc.tile([NB, 1], dt)
            nc.vector.tensor_mul(out=junk_rd[:], in0=key_sb[:], in1=query_sb[:])
            nc.vector.tensor_reduce(out=snew[:], in_=junk_rd[:],
                                    axis=mybir.AxisListType.X, op=mybir.AluOpType.add)

            denom0 = misc.tile([NB, 1], dt)
            neg_m_all = misc.tile([NB, 1], dt)
            attn_groups = []
            g_sb = misc.tile([NB, V], dt)

            def scores_stage(g):
                b0 = g * GRP
                lim = limg[g]
                nA = min(lim, 512)
                nB = max(lim - 512, 0)
                pvmax = pv2[b0]
                tcap = 2 * pvmax

                filled_g = sm_p.tile([GRP, 1], dt, tag="filled_g")
                nc.gpsimd.dma_start(out=filled_g[:], in_=filled_t[b0:b0 + GRP, :])
                penalty_g = sm_p.tile([GRP, S], dt, tag="penalty_g")
                nc.vector.tensor_scalar(
                    out=penalty_g[:], in0=iota_f[:], scalar1=filled_g[:, :1],
                    scalar2=NEG_BIG, op0=mybir.AluOpType.is_ge, op1=mybir.AluOpType.mult)

                # keysT sub-DMAs (2 slots/transfer for group 0 to cut startup)
                ksub = 2 if g == 0 else 4
                kts = []
                for j in range(GRP // ksub):
                    kt = ktile_p.tile([K, 4, S], F8, tag="ktile")
                    nc.gpsimd.dma_start(
                        out=kt[:, :ksub, :lim],
                        in_=kview[:, b0 + ksub * j:b0 + ksub * (j + 1), :lim])
                    kts.append(kt)

                # one-hot query columns: qoh[:, m, c] = qT[:, b0+c] iff m == c
                qoh = grp_p.tile([K, GRP, GRP], F8, tag="qoh")
                nc.vector.memset(qoh[:], 0.0)
                qa = qoh[:, :, :]
                qdiag = bass.AP(tensor=qa.tensor, offset=qa.offset,
                                ap=[qa.ap[0], [GRP + 1, GRP]])
                nc.scalar.copy(out=qdiag, in_=qT_f8[:, b0:b0 + GRP])

                # scores: 16 accumulating fp8 matmuls per 512-col bank
                sc_a = ps_sc.tile([GRP, 512], dt, tag="sc_a")
                for bl in range(GRP):
                    nc.tensor.matmul(out=sc_a[:, :nA], lhsT=qoh[:, bl, :],
                                     rhs=kts[bl // ksub][:, bl % ksub, :nA],
                                     start=(bl == 0), stop=(bl == GRP - 1))
                if nB:
                    sc_b = ps_sc.tile([GRP, 512], dt, tag="sc_b")
                    for bl in range(GRP):
                        nc.tensor.matmul(out=sc_b[:, :nB], lhsT=qoh[:, bl, :],
                                         rhs=kts[bl // ksub][:, bl % ksub, 512:512 + nB],
                                         start=(bl == 0), stop=(bl == GRP - 1))

                # scores + penalty -> SBUF rows; tail past lim is pure penalty
                scores_g = sm_p.tile([GRP, S], dt, tag="scores_g")
                nc.vector.tensor_tensor(out=scores_g[:, :nA], in0=sc_a[:, :nA],
                                        in1=penalty_g[:, :nA], op=mybir.AluOpType.add)
                if nB:
                    nc.vector.tensor_tensor(out=scores_g[:, 512:512 + nB],
                                            in0=sc_b[:, :nB],
                                            in1=penalty_g[:, 512:512 + nB],
                                            op=mybir.AluOpType.add)
                if lim < S:
                    nc.vector.tensor_copy(out=scores_g[:, lim:],
                                          in_=penalty_g[:, lim:])

                m_g = sm_p.tile([GRP, 1], dt, tag="m_g")
                nc.vector.tensor_reduce(out=m_g[:], in_=scores_g[:],
                                        axis=mybir.AxisListType.X,
                                        op=mybir.AluOpType.max)
                # bias = -m/SCALE + ln(128): scales attn x128 into fp8 range
                neg_m_g = sm_p.tile([GRP, 1], dt, tag="neg_m_g")
                nc.vector.tensor_scalar(
                    out=neg_m_g[:], in0=m_g[:], scalar1=-1.0 / SCALE,
                    scalar2=LN_ATT, op0=mybir.AluOpType.mult, op1=mybir.AluOpType.add)
                exps_g = sm_p.tile([GRP, S], dt, tag="exps_g")
                denom0_g = sm_p.tile([GRP, 1], dt, tag="denom0_g")
                nc.scalar.activation(
                    out=exps_g[:], in_=scores_g[:],
                    func=mybir.ActivationFunctionType.Exp,
                    bias=neg_m_g[:, :1], scale=1.0 / SCALE,
                    accum_out=denom0_g[:, :1])

                # one-hot diagonal attn in fp8: aoh[:, t2, i, m, c] nonzero only
                # at m == c (DoubleRow lhsT [128, 2, 16] slices at fixed c)
                aoh = grp_p.tile([128, T2, 2, GRP, GRP], F8, tag="aoh")
                nc.vector.memset(aoh[:, :pvmax], 0.0)
                exps_v = exps_g[:].rearrange("g (t x) -> g t x", x=128)
                for t in range(tcap):
                    tp = ps_tr.tile([128, GRP], dt, tag="tr")
                    nc.tensor.transpose(out=tp[:], in_=exps_v[:, t, :],
                                        identity=identity[:GRP, :GRP])
                    da = aoh[:, t // 2, t % 2, :, :]
                    diag = bass.AP(tensor=da.tensor, offset=da.offset,
                                   ap=[da.ap[0], [GRP + 1, GRP]])
                    nc.scalar.copy(out=diag, in_=tp[:])
                attn_groups.append(aoh)

                nc.gpsimd.dma_start(out=denom0[b0:b0 + GRP, :], in_=denom0_g[:])
                nc.gpsimd.dma_start(out=neg_m_all[b0:b0 + GRP, :], in_=neg_m_g[:])

            def values_stage(g):
                b0 = g * GRP
                aoh = attn_groups[g]
                vts = []
                for j in range(4):
                    pm = pv2[b0 + 4 * j]     # subgroup max (sorted desc)
                    vt = vtile_p.tile([128, 4, T2, 2, V], F8, tag="vtile")
                    nc.sync.dma_start(out=vt[:, :, :pm], in_=vpack_t[b0 + 4 * j:b0 + 4 * j + 4]
                                      .rearrange("b p t i v -> p b t i v")[:, :, :pm])
                    vts.append(vt)
                steps = [(bl, t2) for bl in range(GRP) for t2 in range(pv2[b0 + bl])]
                gw = ps_gw.tile([GRP, V], F32, tag="gw")
                for si, (bl, t2) in enumerate(steps):
                    nc.tensor.matmul(out=gw[:], lhsT=aoh[:, t2, :, :, bl],
                                     rhs=vts[bl // 4][:, bl % 4, t2, :, :],
                                     start=(si == 0), stop=(si == len(steps) - 1),
                                     perf_mode=mybir.MatmulPerfMode.DoubleRow)
                gtmp = grp_p.tile([GRP, V], dt, tag="gtmp")
                nc.scalar.copy(out=gtmp[:], in_=gw[:])
                nc.gpsimd.dma_start(out=g_sb[b0:b0 + GRP, :], in_=gtmp[:])

            # persistent tiles shared by the two tail halves
            wg1T = const.tile([128, 8, V], BF16)
            wg2T = const.tile([128, 4, V], BF16)
            woT = const.tile([128, 4, V], BF16)
            bg1_row = const.tile([1, V], dt)
            bg2_row = const.tile([1, V], dt)
            bo_row = const.tile([1, V], dt)
            eo = misc.tile([NB, 1], dt)
            en = misc.tile([NB, 1], dt)
            mask_wp = misc.tile([NB, 1], dt)
            a_old = misc.tile([NB, 1], dt)
            a_new = misc.tile([NB, 1], dt)
            denom = misc.tile([NB, 1], dt)
            recip = misc.tile([NB, 1], dt)
            t1 = misc.tile([NB, V], dt)
            t2_ = misc.tile([NB, V], dt)
            retr = misc.tile([NB, V], dt)
            retr_bf = misc.tile([NB, V], BF16)
            rT = misc.tile([128, 4, NB], BF16)
            g_act = misc.tile([NB, V], dt)
            g_act_bf = misc.tile([NB, V], BF16)
            gT = misc.tile([128, 4, NB], BF16)
            gate = misc.tile([NB, V], dt)
            z = misc.tile([NB, V], dt)
            z_bf = misc.tile([NB, V], BF16)
            zT = misc.tile([128, 4, NB], BF16)
            out_sb = misc.tile([NB, V], dt)

            def load_mlp_weights():
                nc.scalar.dma_start(out=wg1T[:], in_=wg1T_t[:].rearrange("(c p) j -> p c j", p=128))
                nc.scalar.dma_start(out=wg2T[:], in_=wg2T_t[:].rearrange("(c p) j -> p c j", p=128))
                nc.scalar.dma_start(out=woT[:], in_=woT_t[:].rearrange("(c p) j -> p c j", p=128))
                nc.scalar.dma_start(out=bg1_row[:], in_=bg1_t[None, :])
                nc.scalar.dma_start(out=bg2_row[:], in_=bg2_t[None, :])
                nc.scalar.dma_start(out=bo_row[:], in_=bo_t[None, :])

            def tail_half(h0):
                """corrections + MLP for batches [h0, h0+32); h0 in {0, 32}."""
                H = NB // 2
                hs = slice(h0, h0 + H)
                idb = identity_bf[hs, hs]   # 32x32 identity block at base h0
                nc.scalar.activation(out=eo[hs], in_=sold[hs],
                                     func=mybir.ActivationFunctionType.Exp,
                                     bias=neg_m_all[hs, :1], scale=1.0 / SCALE)
                nc.scalar.activation(out=en[hs], in_=snew[hs],
                                     func=mybir.ActivationFunctionType.Exp,
                                     bias=neg_m_all[hs, :1], scale=1.0 / SCALE)
                nc.vector.tensor_tensor(out=mask_wp[hs], in0=wp_sb[hs],
                                        in1=filled_sb[hs], op=mybir.AluOpType.is_lt)
                nc.vector.tensor_mul(out=a_old[hs], in0=eo[hs], in1=mask_wp[hs])
                nc.vector.tensor_mul(out=a_new[hs], in0=en[hs], in1=mask_wp[hs])
                nc.vector.tensor_sub(out=denom[hs], in0=denom0[hs], in1=a_old[hs])
                nc.vector.tensor_add(out=denom[hs], in0=denom[hs], in1=a_new[hs])
                nc.vector.reciprocal(out=recip[hs], in_=denom[hs])

                nc.vector.tensor_scalar_mul(out=t1[hs], in0=value_sb[hs],
                                            scalar1=a_new[hs, :1])
                nc.vector.tensor_scalar_mul(out=t2_[hs], in0=vwp_sb[hs],
                                            scalar1=a_old[hs, :1])
                nc.vector.tensor_sub(out=t1[hs], in0=t1[hs], in1=t2_[hs])
                nc.vector.tensor_add(out=t1[hs], in0=g_sb[hs], in1=t1[hs])
                nc.vector.tensor_scalar_mul(out=retr[hs], in0=t1[hs],
                                            scalar1=recip[hs, :1])

                nc.vector.tensor_copy(out=retr_bf[hs], in_=retr[hs])
                for c in range(4):
                    tp = ps_tr.tile([128, H], BF16, tag="tr")
                    nc.tensor.transpose(out=tp[:], in_=retr_bf[hs, c * 128:(c + 1) * 128],
                                        identity=idb)
                    nc.scalar.copy(out=rT[:, c, h0:h0 + H], in_=tp[:])

                g_ps = ps_tr.tile([NB, V], dt, tag="tr")
                for ic in range(8):
                    lhsT = hT[:, ic, h0:h0 + H] if ic < 4 else rT[:, ic - 4, h0:h0 + H]
                    nc.tensor.matmul(out=g_ps[hs, :], lhsT=lhsT, rhs=wg1T[:, ic, :],
                                     start=(ic == 0), stop=False)
                nc.tensor.matmul(out=g_ps[hs, :], lhsT=ones_row[:, :H], rhs=bg1_row[:],
                                 start=False, stop=True)
                nc.scalar.activation(out=g_act[hs], in_=g_ps[hs, :],
                                     func=mybir.ActivationFunctionType.Sigmoid)
                nc.vector.tensor_mul(out=g_act[hs], in0=g_act[hs], in1=g_ps[hs, :])

                nc.vector.tensor_copy(out=g_act_bf[hs], in_=g_act[hs])
                for c in range(4):
                    tp = ps_tr.tile([128, H], BF16, tag="tr")
                    nc.tensor.transpose(out=tp[:], in_=g_act_bf[hs, c * 128:(c + 1) * 128],
                                        identity=idb)
                    nc.scalar.copy(out=gT[:, c, h0:h0 + H], in_=tp[:])

                gate_ps = ps_tr.tile([NB, V], dt, tag="tr")
                for c in range(4):
                    nc.tensor.matmul(out=gate_ps[hs, :], lhsT=gT[:, c, h0:h0 + H],
                                     rhs=wg2T[:, c, :], start=(c == 0), stop=False)
                nc.tensor.matmul(out=gate_ps[hs, :], lhsT=ones_row[:, :H],
                                 rhs=bg2_row[:], start=False, stop=True)
                nc.scalar.activation(out=gate[hs], in_=gate_ps[hs, :],
                                     func=mybir.ActivationFunctionType.Sigmoid)

                nc.vector.tensor_mul(out=z[hs], in0=gate[hs], in1=retr[hs])
                nc.vector.tensor_add(out=z[hs], in0=z[hs], in1=hidden_sb[hs])
                nc.vector.tensor_copy(out=z_bf[hs], in_=z[hs])
                for c in range(4):
                    tp = ps_tr.tile([128, H], BF16, tag="tr")
                    nc.tensor.transpose(out=tp[:], in_=z_bf[hs, c * 128:(c + 1) * 128],
                                        identity=idb)
                    nc.scalar.copy(out=zT[:, c, h0:h0 + H], in_=tp[:])

                out_ps = ps_tr.tile([NB, V], dt, tag="tr")
                for c in range(4):
                    nc.tensor.matmul(out=out_ps[hs, :], lhsT=zT[:, c, h0:h0 + H],
                                     rhs=woT[:, c, :], start=(c == 0), stop=False)
                nc.tensor.matmul(out=out_ps[hs, :], lhsT=ones_row[:, :H],
                                 rhs=bo_row[:], start=False, stop=True)
                nc.vector.tensor_copy(out=out_sb[hs], in_=out_ps[hs, :])
                nc.sync.dma_start(out=out_t[hs, :], in_=out_sb[hs])

            scores_stage(0)
            values_stage(0)
            load_mlp_weights()
            scores_stage(1)
            values_stage(1)
            tail_half(0)        # overlaps groups 2-3
            scores_stage(2)
            values_stage(2)
            scores_stage(3)
            values_stage(3)
            tail_half(NB // 2)

    nc.finalize()
    return nc


_NC_CACHE = {}


def _get_nc(pv2, limg):
    key = (tuple(pv2), tuple(limg))
    if key not in _NC_CACHE:
        _NC_CACHE.clear()
        _NC_CACHE[key] = _build(tuple(pv2), tuple(limg))
    return _NC_CACHE[key]


def _make_plan(filled):
    fl = np.asarray(filled).astype(np.int64)
    f_w = np.minimum(fl + 1, S)
    order = np.argsort(-f_w, kind="stable")
    idx = order.reshape(NB, NCORES)          # slot i, core c -> batch idx[i, c]
    fmax = f_w[idx[:, 0]]
    pv2 = np.minimum((fmax + 255) // 256, T2).astype(np.int64)
    limg = tuple(int(min((fmax[g * GRP] + 7) // 8 * 8, S)) for g in range(NG))
    return idx, tuple(int(x) for x in pv2), limg


def _make_in_maps(idx, keys, values, key, value, hidden, write_ptr, filled,
                  Wq, bq, Wg1, bg1, Wg2, bg2, Wo, bo):
    f32 = np.float32
    bidx = np.arange(B)
    wp = np.asarray(write_ptr).astype(np.int64)
    fl = np.asarray(filled).astype(np.int64)

    keys_f8 = np.asarray(keys, dtype=f32).astype(NP_F8)
    kwp = keys_f8[bidx, wp].astype(f32)
    keysT = np.ascontiguousarray(keys_f8.transpose(0, 2, 1))      # [B, K, S]

    values_f8 = np.asarray(values, dtype=f32).astype(NP_F8)
    vwp = values_f8[bidx, wp].astype(f32)
    # vpack[b, p, t2, i, v] = values[b, t2*256 + i*128 + p, v]
    vpack = np.ascontiguousarray(
        values_f8.reshape(B, T2, 2, 128, V).transpose(0, 3, 1, 2, 4))

    key = np.asarray(key, dtype=f32)
    value = np.asarray(value, dtype=f32)
    hidden = np.asarray(hidden, dtype=f32)

    wqT = np.ascontiguousarray(np.asarray(Wq, dtype=f32).T).astype(NP_BF16)
    wg1T = np.ascontiguousarray(np.asarray(Wg1, dtype=f32).T).astype(NP_BF16)
    wg2T = np.ascontiguousarray(np.asarray(Wg2, dtype=f32).T).astype(NP_BF16)
    woT = np.ascontiguousarray(np.asarray(Wo, dtype=f32).T).astype(NP_BF16)
    bq = np.ascontiguousarray(np.asarray(bq, dtype=f32))
    bg1 = np.ascontiguousarray(np.asarray(bg1, dtype=f32))
    bg2 = np.ascontiguousarray(np.asarray(bg2, dtype=f32))
    bo = np.ascontiguousarray(np.asarray(bo, dtype=f32))

    filled_w = np.minimum(fl + 1, S).astype(f32).reshape(B, 1)
    wp_f = wp.astype(f32).reshape(B, 1)

    in_maps = []
    for c in range(NCORES):
        sel = idx[:, c]
        in_maps.append({
            "keysT": np.ascontiguousarray(keysT[sel]),
            "vpack": np.ascontiguousarray(vpack[sel]),
            "key": np.ascontiguousarray(key[sel]),
            "value": np.ascontiguousarray(value[sel]),
            "hidden": np.ascontiguousarray(hidden[sel]),
            "filled_f": np.ascontiguousarray(filled_w[sel]),
            "wp_f": np.ascontiguousarray(wp_f[sel]),
            "kwp": np.ascontiguousarray(kwp[sel]),
            "vwp": np.ascontiguousarray(vwp[sel]),
            "WqT": wqT, "Wg1T": wg1T, "Wg2T": wg2T, "WoT": woT,
            "bq": bq, "bg1": bg1, "bg2": bg2, "bo": bo,
        })
    return in_maps


def run(trace=False, **inputs):
    idx, pv2, limg = _make_plan(inputs["filled"])
    nc = _get_nc(pv2, limg)
    in_maps = _make_in_maps(idx, **inputs)
    res = run_bass_kernel_spmd(nc, in_maps, core_ids=list(range(NCORES)),
                               trace=trace)
    out = np.empty((B, V), np.float32)
    for c in range(NCORES):
        out[idx[:, c]] = res.results[c]["out"]
    return out, res


def kernel(**inputs) -> np.ndarray:
    out, _ = run(trace=False, **inputs)
    return out


# revision 21
# speedup vs baseline: 1.0255x; 1.0255x over previous
"""EpisodicMemory Trainium2 kernel (8 NeuronCores, pure data parallel over batch).

Reference semantics (per batch b):
    keys_w   = keys   with row write_ptr[b] <- key[b]
    values_w = values with row write_ptr[b] <- value[b]
    filled_w = min(filled + 1, S)
    query    = hidden @ Wq.T + bq
    scores   = (keys_w @ query) / sqrt(K), masked to s < filled_w
    attn     = softmax(scores)
    retrieved= attn @ values_w
    g        = silu([hidden|retrieved] @ Wg1.T + bg1)
    gate     = sigmoid(g @ Wg2.T + bg2)
    out      = (hidden + gate*retrieved) @ Wo.T + bo

The scatter is never materialized: base scores/retrieved are computed from the
original keys/values and corrected algebraically with the old rows at
write_ptr (host-gathered) plus the new key/value rows.

v3 design:
  * keys host-transposed to [K, S] bf16 -> scores are PE matmuls (contract K).
    16 batches accumulate into one [16, 512] PSUM bank via one-hot query
    columns (out partition offsets must be 0 mod 32, so rows are selected by
    zero-padding the stationary operand instead).
  * values in fp8e4, host-packed in (s%128, s//256, (s//128)%2, v) order for
    perf_mode=DoubleRow matmuls (2 fp8 rows per PE cell); attention weights
    are scaled by 128 into fp8 range (denominator scales identically so the
    softmax normalization cancels the factor).  One-hot diagonal layout lets
    all 16 batches of a group accumulate into one [16, 512] PSUM bank.
  * rows s >= filled_w never contribute (scores masked to -inf), so slot i
    only reads/computes ceil-rounded row counts baked from the host-sorted
    filled profile: batches sorted by filled_w desc, rank 8i+c -> core c
    slot i, so all 8 cores share one compiled program.
"""

import sys

sys.path.insert(0, "/opt/trn_rl_repo")

import numpy as np
import ml_dtypes

import concourse.bacc as bacc
import concourse.tile as tile
from concourse import bass, mybir
from concourse.bass_utils import run_bass_kernel_spmd
from concourse.masks import make_identity

B, S, K, V = 512, 1024, 128, 512
NCORES = 8
NB = B // NCORES          # 64 batches per core
T2 = S // 256             # 4 value double-chunks of 256 rows
GRP = 16                  # batches per softmax group
NG = NB // GRP            # 4 groups
SCALE = float(np.sqrt(K))
NEG_BIG = -3.0e37
LN_ATT = float(np.log(128.0))   # attn weights scaled x128 into fp8 range

F32 = mybir.dt.float32
BF16 = mybir.dt.bfloat16
F8 = mybir.dt.float8e4
NP_BF16 = np.dtype(ml_dtypes.bfloat16)
NP_F8 = np.dtype(ml_dtypes.float8_e4m3)


def _build(pv2, limg):
    """pv2[i]: value 256-row double-chunks for slot i (1..4); limg[g]: key rows
    (multiple of 8) read/scored for group g.  Slots sorted descending."""
    nc = bacc.Bacc()
    dt = F32
    DR = mybir.MatmulPerfMode.DoubleRow

    keysT_t = nc.dram_tensor("keysT", [NB, K, S], F8, kind="ExternalInput")
    vpack_t = nc.dram_tensor("vpack", [NB, 128, T2, 2, V], F8, kind="ExternalInput")
    key_t = nc.dram_tensor("key", [NB, K], dt, kind="ExternalInput")
    value_t = nc.dram_tensor("value", [NB, V], dt, kind="ExternalInput")
    hidden_t = nc.dram_tensor("hidden", [NB, V], dt, kind="ExternalInput")
    filled_t = nc.dram_tensor("filled_f", [NB, 1], dt, kind="ExternalInput")
    wp_t = nc.dram_tensor("wp_f", [NB, 1], dt, kind="ExternalInput")
    kwp_t = nc.dram_tensor("kwp", [NB, K], dt, kind="ExternalInput")
    vwp_t = nc.dram_tensor("vwp", [NB, V], dt, kind="ExternalInput")
    wqT_t = nc.dram_tensor("WqT", [V, K], BF16, kind="ExternalInput")
    wg1T_t = nc.dram_tensor("Wg1T", [2 * V, V], BF16, kind="ExternalInput")
    wg2T_t = nc.dram_tensor("Wg2T", [V, V], BF16, kind="ExternalInput")
    woT_t = nc.dram_tensor("WoT", [V, V], BF16, kind="ExternalInput")
    bq_t = nc.dram_tensor("bq", [K], BF16, kind="ExternalInput")
    bg1_t = nc.dram_tensor("bg1", [V], BF16, kind="ExternalInput")
    bg2_t = nc.dram_tensor("bg2", [V], BF16, kind="ExternalInput")
    bo_t = nc.dram_tensor("bo", [V], BF16, kind="ExternalInput")
    out_t = nc.dram_tensor("out", [NB, V], dt, kind="ExternalOutput")

    kview = keysT_t[:].rearrange("b k s -> k b s")

    with tile.TileContext(nc) as tc:
        with (
            tc.tile_pool(name="const", bufs=1) as const,
            tc.tile_pool(name="ktile", bufs=5) as ktile_p,
            tc.tile_pool(name="vtile", bufs=5) as vtile_p,
            tc.tile_pool(name="grp", bufs=2) as grp_p,
            tc.tile_pool(name="sm", bufs=1) as sm_p,
            tc.tile_pool(name="misc", bufs=1) as misc,
            tc.tile_pool(name="ps_sc", bufs=2, space="PSUM") as ps_sc,
            tc.tile_pool(name="ps_gw", bufs=2, space="PSUM") as ps_gw,
            tc.tile_pool(name="ps_tr", bufs=2, space="PSUM") as ps_tr,
        ):
            # ---------------- setup ----------------
            hidden_sb = misc.tile([NB, V], dt)
            nc.scalar.dma_start(out=hidden_sb[:], in_=hidden_t[:, :])
            wqT = const.tile([128, 4, K], BF16)
            nc.scalar.dma_start(out=wqT[:], in_=wqT_t[:].rearrange("(c p) k -> p c k", p=128))
            bq_row = const.tile([1, K], BF16)
            nc.scalar.dma_start(out=bq_row[:], in_=bq_t[None, :])

            identity = const.tile([128, 128], dt)
            make_identity(nc, identity[:])
            identity_bf = const.tile([128, 128], BF16)
            nc.vector.tensor_copy(out=identity_bf[:], in_=identity[:])
            ones_row = const.tile([1, 128], dt)
            nc.vector.memset(ones_row[:], 1.0)
            ones_bf = const.tile([1, 128], BF16)
            nc.vector.memset(ones_bf[:], 1.0)

            iota_i = misc.tile([GRP, S], mybir.dt.int16)
            nc.gpsimd.iota(iota_i[:], pattern=[[1, S]], base=0, channel_multiplier=0)
            iota_f = const.tile([GRP, S], dt)
            nc.vector.tensor_copy(out=iota_f[:], in_=iota_i[:])

            key_sb = misc.tile([NB, K], dt)
            nc.scalar.dma_start(out=key_sb[:], in_=key_t[:, :])
            value_sb = misc.tile([NB, V], dt)
            nc.scalar.dma_start(out=value_sb[:], in_=value_t[:, :])
            filled_sb = misc.tile([NB, 1], dt)
            nc.scalar.dma_start(out=filled_sb[:], in_=filled_t[:, :])
            wp_sb = misc.tile([NB, 1], dt)
            nc.scalar.dma_start(out=wp_sb[:], in_=wp_t[:, :])
            kwp_sb = misc.tile([NB, K], dt)
            nc.scalar.dma_start(out=kwp_sb[:], in_=kwp_t[:, :])
            vwp_sb = misc.tile([NB, V], dt)
            nc.scalar.dma_start(out=vwp_sb[:], in_=vwp_t[:, :])

            # hidden in bf16 + hiddenT (128v x 64b) chunks for the matmuls
            hidden_bf = misc.tile([NB, V], BF16)
            nc.vector.tensor_copy(out=hidden_bf[:], in_=hidden_sb[:])
            hT = misc.tile([128, 4, NB], BF16)
            for c in range(4):
                tp = ps_tr.tile([128, NB], BF16, tag="tr")
                nc.tensor.transpose(out=tp[:], in_=hidden_bf[:, c * 128:(c + 1) * 128],
                                    identity=identity_bf[:NB, :NB])
                nc.scalar.copy(out=hT[:, c, :], in_=tp[:])

            # query both ways: qT (128k x 64b) for scores, q (64b x 128k) for
            # the write-row correction dot products
            qT_ps = ps_tr.tile([K, NB], dt, tag="tr")
            for c in range(4):
                nc.tensor.matmul(out=qT_ps[:], lhsT=wqT[:, c, :], rhs=hT[:, c, :],
                                 start=(c == 0), stop=False)
            nc.tensor.matmul(out=qT_ps[:], lhsT=bq_row[:], rhs=ones_bf[:, :NB],
                             start=False, stop=True)
            qT_f8 = misc.tile([K, NB], F8)
            nc.scalar.copy(out=qT_f8[:], in_=qT_ps[:])

            q_ps = ps_tr.tile([NB, K], dt, tag="tr")
            for c in range(4):
                nc.tensor.matmul(out=q_ps[:], lhsT=hT[:, c, :], rhs=wqT[:, c, :],
                                 start=(c == 0), stop=False)
            nc.tensor.matmul(out=q_ps[:], lhsT=ones_bf[:, :NB], rhs=bq_row[:],
                             start=False, stop=True)
            query_sb = misc.tile([NB, K], dt)
            nc.vector.tensor_copy(out=query_sb[:], in_=q_ps[:])

            junk_rd = misc.tile([NB, K], dt)
            sold = misc.tile([NB, 1], dt)
            nc.vector.tensor_mul(out=junk_rd[:], in0=kwp_sb[:], in1=query_sb[:])
            nc.vector.tensor_reduce(out=sold[:], in_=junk_rd[:],
                                    axis=mybir.AxisListType.X, op=mybir.AluOpType.add)
            snew = misc.tile([NB, 1], dt)
            nc.vector.tensor_mul(out=junk_rd[:], in0=key_sb[:], in1=query_sb[:])
            nc.vector.tensor_reduce(out=snew[:], in_=junk_rd[:],
                                    axis=mybir.AxisListType.X, op=mybir.AluOpType.add)

            denom0 = misc.tile([NB, 1], dt)
            neg_m_all = misc.tile([NB, 1], dt)
            attn_groups = []
            g_sb = misc.tile([NB, V], dt)

            def scores_stage(g):
                b0 = g * GRP
                lim = limg[g]
                nA = min(lim, 512)
                nB = max(lim - 512, 0)
                pvmax = pv2[b0]
                tcap = 2 * pvmax

                filled_g = sm_p.tile([GRP, 1], dt, tag="filled_g")
                nc.gpsimd.dma_start(out=filled_g[:], in_=filled_t[b0:b0 + GRP, :])
                penalty_g = sm_p.tile([GRP, S], dt, tag="penalty_g")
                nc.vector.tensor_scalar(
                    out=penalty_g[:], in0=iota_f[:], scalar1=filled_g[:, :1],
                    scalar2=NEG_BIG, op0=mybir.AluOpType.is_ge, op1=mybir.AluOpType.mult)

                # keysT sub-DMAs (2 slots/transfer for group 0 to cut startup)
                ksub = 2 if g == 0 else 4
                kts = []
                for j in range(GRP // ksub):
                    kt = ktile_p.tile([K, 4, S], F8, tag="ktile")
                    nc.gpsimd.dma_start(
                        out=kt[:, :ksub, :lim],
                        in_=kview[:, b0 + ksub * j:b0 + ksub * (j + 1), :lim])
                    kts.append(kt)

                # one-hot query columns: qoh[:, m, c] = qT[:, b0+c] iff m == c
                qoh = grp_p.tile([K, GRP, GRP], F8, tag="qoh")
                nc.vector.memset(qoh[:], 0.0)
                qa = qoh[:, :, :]
                qdiag = bass.AP(tensor=qa.tensor, offset=qa.offset,
                                ap=[qa.ap[0], [GRP + 1, GRP]])
                nc.scalar.copy(out=qdiag, in_=qT_f8[:, b0:b0 + GRP])

                # scores: 16 accumulating fp8 matmuls per 512-col bank
                sc_a = ps_sc.tile([GRP, 512], dt, tag="sc_a")
                for bl in range(GRP):
                    nc.tensor.matmul(out=sc_a[:, :nA], lhsT=qoh[:, bl, :],
                                     rhs=kts[bl // ksub][:, bl % ksub, :nA],
                                     start=(bl == 0), stop=(bl == GRP - 1))
                if nB:
                    sc_b = ps_sc.tile([GRP, 512], dt, tag="sc_b")
                    for bl in range(GRP):
                        nc.tensor.matmul(out=sc_b[:, :nB], lhsT=qoh[:, bl, :],
                                         rhs=kts[bl // ksub][:, bl % ksub, 512:512 + nB],
                                         start=(bl == 0), stop=(bl == GRP - 1))

                # scores + penalty -> SBUF rows; tail past lim is pure penalty
                scores_g = sm_p.tile([GRP, S], dt, tag="scores_g")
                nc.vector.tensor_tensor(out=scores_g[:, :nA], in0=sc_a[:, :nA],
                                        in1=penalty_g[:, :nA], op=mybir.AluOpType.add)
                if nB:
                    nc.vector.tensor_tensor(out=scores_g[:, 512:512 + nB],
                                            in0=sc_b[:, :nB],
                                            in1=penalty_g[:, 512:512 + nB],
                                            op=mybir.AluOpType.add)
                if lim < S:
                    nc.vector.tensor_copy(out=scores_g[:, lim:],
                                          in_=penalty_g[:, lim:])

                m_g = sm_p.tile([GRP, 1], dt, tag="m_g")
                nc.vector.tensor_reduce(out=m_g[:], in_=scores_g[:],
                                        axis=mybir.AxisListType.X,
                                        op=mybir.AluOpType.max)
                # bias = -m/SCALE + ln(128): scales attn x128 into fp8 range
                neg_m_g = sm_p.tile([GRP, 1], dt, tag="neg_m_g")
                nc.vector.tensor_scalar(
                    out=neg_m_g[:], in0=m_g[:], scalar1=-1.0 / SCALE,
                    scalar2=LN_ATT, op0=mybir.AluOpType.mult, op1=mybir.AluOpType.add)
                exps_g = sm_p.tile([GRP, S], dt, tag="exps_g")
                denom0_g = sm_p.tile([GRP, 1], dt, tag="denom0_g")
                nc.scalar.activation(
                    out=exps_g[:], in_=scores_g[:],
                    func=mybir.ActivationFunctionType.Exp,
                    bias=neg_m_g[:, :1], scale=1.0 / SCALE,
                    accum_out=denom0_g[:, :1])

                # one-hot diagonal attn in fp8: aoh[:, t2, i, m, c] nonzero only
                # at m == c (DoubleRow lhsT [128, 2, 16] slices at fixed c)
                aoh = grp_p.tile([128, T2, 2, GRP, GRP], F8, tag="aoh")
                nc.vector.memset(aoh[:, :pvmax], 0.0)
                exps_v = exps_g[:].rearrange("g (t x) -> g t x", x=128)
                for t in range(tcap):
                    tp = ps_tr.tile([128, GRP], dt, tag="tr")
                    nc.tensor.transpose(out=tp[:], in_=exps_v[:, t, :],
                                        identity=identity[:GRP, :GRP])
                    da = aoh[:, t // 2, t % 2, :, :]
                    diag = bass.AP(tensor=da.tensor, offset=da.offset,
                                   ap=[da.ap[0], [GRP + 1, GRP]])
                    nc.scalar.copy(out=diag, in_=tp[:])
                attn_groups.append(aoh)

                nc.gpsimd.dma_start(out=denom0[b0:b0 + GRP, :], in_=denom0_g[:])
                nc.gpsimd.dma_start(out=neg_m_all[b0:b0 + GRP, :], in_=neg_m_g[:])

            def values_stage(g):
                b0 = g * GRP
                aoh = attn_groups[g]
                vts = []
                for j in range(4):
                    pm = pv2[b0 + 4 * j]     # subgroup max (sorted desc)
                    vt = vtile_p.tile([128, 4, T2, 2, V], F8, tag="vtile")
                    nc.sync.dma_start(out=vt[:, :, :pm], in_=vpack_t[b0 + 4 * j:b0 + 4 * j + 4]
                                      .rearrange("b p t i v -> p b t i v")[:, :, :pm])
                    vts.append(vt)
                steps = [(bl, t2) for bl in range(GRP) for t2 in range(pv2[b0 + bl])]
                gw = ps_gw.tile([GRP, V], F32, tag="gw")
                for si, (bl, t2) in enumerate(steps):
                    nc.tensor.matmul(out=gw[:], lhsT=aoh[:, t2, :, :, bl],
                                     rhs=vts[bl // 4][:, bl % 4, t2, :, :],
                                     start=(si == 0), stop=(si == len(steps) - 1),
                                     perf_mode=mybir.MatmulPerfMode.DoubleRow)
                gtmp = grp_p.tile([GRP, V], dt, tag="gtmp")
                nc.scalar.copy(out=gtmp[:], in_=gw[:])
                nc.gpsimd.dma_start(out=g_sb[b0:b0 + GRP, :], in_=gtmp[:])

            # persistent tiles shared by the two tail halves
            wg1T = const.tile([128, 8, V], BF16)
            wg2T = const.tile([128, 4, V], BF16)
            woT = const.tile([128, 4, V], BF16)
            bg1_row = const.tile([1, V], BF16)
            bg2_row = const.tile([1, V], BF16)
            bo_row = const.tile([1, V], BF16)
            eo = misc.tile([NB, 1], dt)
            en = misc.tile([NB, 1], dt)
            mask_wp = misc.tile([NB, 1], dt)
            a_old = misc.tile([NB, 1], dt)
            a_new = misc.tile([NB, 1], dt)
            denom = misc.tile([NB, 1], dt)
            recip = misc.tile([NB, 1], dt)
            t1 = misc.tile([NB, V], dt)
            t2_ = misc.tile([NB, V], dt)
            retr = misc.tile([NB, V], dt)
            retr_bf = misc.tile([NB, V], BF16)
            rT = misc.tile([128, 4, NB], BF16)
            g_act = misc.tile([NB, V], dt)
            g_act_bf = misc.tile([NB, V], BF16)
            gT = misc.tile([128, 4, NB], BF16)
            gate = misc.tile([NB, V], dt)
            z = misc.tile([NB, V], dt)
            z_bf = misc.tile([NB, V], BF16)
            zT = misc.tile([128, 4, NB], BF16)
            out_sb = misc.tile([NB, V], dt)

            def load_mlp_weights():
                nc.scalar.dma_start(out=wg1T[:], in_=wg1T_t[:].rearrange("(c p) j -> p c j", p=128))
                nc.scalar.dma_start(out=wg2T[:], in_=wg2T_t[:].rearrange("(c p) j -> p c j", p=128))
                nc.scalar.dma_start(out=woT[:], in_=woT_t[:].rearrange("(c p) j -> p c j", p=128))
                nc.scalar.dma_start(out=bg1_row[:], in_=bg1_t[None, :])
                nc.scalar.dma_start(out=bg2_row[:], in_=bg2_t[None, :])
                nc.scalar.dma_start(out=bo_row[:], in_=bo_t[None, :])

            def exps_half(h0):
                """write-row correction exps for batches [h0, h0+32)."""
                H = NB // 2
                hs = slice(h0, h0 + H)
                nc.scalar.activation(out=eo[hs], in_=sold[hs],
                                     func=mybir.ActivationFunctionType.Exp,
                                     bias=neg_m_all[hs, :1], scale=1.0 / SCALE)
                nc.scalar.activation(out=en[hs], in_=snew[hs],
                                     func=mybir.ActivationFunctionType.Exp,
                                     bias=neg_m_all[hs, :1], scale=1.0 / SCALE)

            def corr_half(h0):
                """denominator + retrieved for batches [h0, h0+32) (DVE)."""
                H = NB // 2
                hs = slice(h0, h0 + H)
                nc.vector.tensor_tensor(out=mask_wp[hs], in0=wp_sb[hs],
                                        in1=filled_sb[hs], op=mybir.AluOpType.is_lt)
                nc.vector.tensor_mul(out=a_old[hs], in0=eo[hs], in1=mask_wp[hs])
                nc.vector.tensor_mul(out=a_new[hs], in0=en[hs], in1=mask_wp[hs])
                nc.vector.tensor_sub(out=denom[hs], in0=denom0[hs], in1=a_old[hs])
                nc.vector.tensor_add(out=denom[hs], in0=denom[hs], in1=a_new[hs])
                nc.vector.reciprocal(out=recip[hs], in_=denom[hs])

                nc.vector.tensor_scalar_mul(out=t1[hs], in0=value_sb[hs],
                                            scalar1=a_new[hs, :1])
                nc.vector.tensor_scalar_mul(out=t2_[hs], in0=vwp_sb[hs],
                                            scalar1=a_old[hs, :1])
                nc.vector.tensor_sub(out=t1[hs], in0=t1[hs], in1=t2_[hs])
                nc.vector.tensor_add(out=t1[hs], in0=g_sb[hs], in1=t1[hs])
                nc.vector.tensor_scalar_mul(out=retr[hs], in0=t1[hs],
                                            scalar1=recip[hs, :1])
                nc.vector.tensor_copy(out=retr_bf[hs], in_=retr[hs])

            def mlp_half(h0):
                """gated MLP for batches [h0, h0+32) (PE-heavy)."""
                H = NB // 2
                hs = slice(h0, h0 + H)
                idb = identity_bf[hs, hs]   # 32x32 identity block at base h0
                for c in range(4):
                    tp = ps_tr.tile([128, H], BF16, tag="tr")
                    nc.tensor.transpose(out=tp[:], in_=retr_bf[hs, c * 128:(c + 1) * 128],
                                        identity=idb)
                    nc.scalar.copy(out=rT[:, c, h0:h0 + H], in_=tp[:])

                g_ps = ps_tr.tile([NB, V], dt, tag="tr")
                for ic in range(8):
                    lhsT = hT[:, ic, h0:h0 + H] if ic < 4 else rT[:, ic - 4, h0:h0 + H]
                    nc.tensor.matmul(out=g_ps[hs, :], lhsT=lhsT, rhs=wg1T[:, ic, :],
                                     start=(ic == 0), stop=False)
                nc.tensor.matmul(out=g_ps[hs, :], lhsT=ones_bf[:, :H], rhs=bg1_row[:],
                                 start=False, stop=True)
                nc.scalar.activation(out=g_act[hs], in_=g_ps[hs, :],
                                     func=mybir.ActivationFunctionType.Sigmoid)
                nc.vector.tensor_mul(out=g_act[hs], in0=g_act[hs], in1=g_ps[hs, :])

                nc.vector.tensor_copy(out=g_act_bf[hs], in_=g_act[hs])
                for c in range(4):
                    tp = ps_tr.tile([128, H], BF16, tag="tr")
                    nc.tensor.transpose(out=tp[:], in_=g_act_bf[hs, c * 128:(c + 1) * 128],
                                        identity=idb)
                    nc.scalar.copy(out=gT[:, c, h0:h0 + H], in_=tp[:])

                gate_ps = ps_tr.tile([NB, V], dt, tag="tr")
                for c in range(4):
                    nc.tensor.matmul(out=gate_ps[hs, :], lhsT=gT[:, c, h0:h0 + H],
                                     rhs=wg2T[:, c, :], start=(c == 0), stop=False)
                nc.tensor.matmul(out=gate_ps[hs, :], lhsT=ones_bf[:, :H],
                                 rhs=bg2_row[:], start=False, stop=True)
                nc.scalar.activation(out=gate[hs], in_=gate_ps[hs, :],
                                     func=mybir.ActivationFunctionType.Sigmoid)

                nc.vector.tensor_mul(out=z[hs], in0=gate[hs], in1=retr[hs])
                nc.vector.tensor_add(out=z[hs], in0=z[hs], in1=hidden_sb[hs])
                nc.vector.tensor_copy(out=z_bf[hs], in_=z[hs])
                for c in range(4):
                    tp = ps_tr.tile([128, H], BF16, tag="tr")
                    nc.tensor.transpose(out=tp[:], in_=z_bf[hs, c * 128:(c + 1) * 128],
                                        identity=idb)
                    nc.scalar.copy(out=zT[:, c, h0:h0 + H], in_=tp[:])

                out_ps = ps_tr.tile([NB, V], dt, tag="tr")
                for c in range(4):
                    nc.tensor.matmul(out=out_ps[hs, :], lhsT=zT[:, c, h0:h0 + H],
                                     rhs=woT[:, c, :], start=(c == 0), stop=False)
                nc.tensor.matmul(out=out_ps[hs, :], lhsT=ones_bf[:, :H],
                                 rhs=bo_row[:], start=False, stop=True)
                nc.vector.tensor_copy(out=out_sb[hs], in_=out_ps[hs, :])
                nc.sync.dma_start(out=out_t[hs, :], in_=out_sb[hs])

            H2 = NB // 2
            scores_stage(0)
            values_stage(0)
            load_mlp_weights()
            scores_stage(1)
            values_stage(1)
            exps_half(0)        # Exp table loaded while PE runs scores(2)
            corr_half(0)        # DVE-only, overlaps scores(2)/values(2)
            scores_stage(2)
            values_stage(2)
            mlp_half(0)         # PE work with inputs long ready
            scores_stage(3)
            exps_half(H2)       # needs neg_m of groups 2-3 only
            values_stage(3)
            corr_half(H2)
            mlp_half(H2)

    nc.finalize()
    return nc


_NC_CACHE = {}


def _get_nc(pv2, limg):
    key = (tuple(pv2), tuple(limg))
    if key not in _NC_CACHE:
        _NC_CACHE.clear()
        _NC_CACHE[key] = _build(tuple(pv2), tuple(limg))
    return _NC_CACHE[key]


def _make_plan(filled):
    fl = np.asarray(filled).astype(np.int64)
    f_w = np.minimum(fl + 1, S)
    order = np.argsort(-f_w, kind="stable")
    idx = order.reshape(NB, NCORES)          # slot i, core c -> batch idx[i, c]
    fmax = f_w[idx[:, 0]]
    pv2 = np.minimum((fmax + 255) // 256, T2).astype(np.int64)
    limg = tuple(int(min((fmax[g * GRP] + 7) // 8 * 8, S)) for g in range(NG))
    return idx, tuple(int(x) for x in pv2), limg


def _make_in_maps(idx, keys, values, key, value, hidden, write_ptr, filled,
                  Wq, bq, Wg1, bg1, Wg2, bg2, Wo, bo):
    f32 = np.float32
    bidx = np.arange(B)
    wp = np.asarray(write_ptr).astype(np.int64)
    fl = np.asarray(filled).astype(np.int64)

    keys_f8 = np.asarray(keys, dtype=f32).astype(NP_F8)
    kwp = keys_f8[bidx, wp].astype(f32)
    keysT = np.ascontiguousarray(keys_f8.transpose(0, 2, 1))      # [B, K, S]

    values_f8 = np.asarray(values, dtype=f32).astype(NP_F8)
    vwp = values_f8[bidx, wp].astype(f32)
    # vpack[b, p, t2, i, v] = values[b, t2*256 + i*128 + p, v]
    vpack = np.ascontiguousarray(
        values_f8.reshape(B, T2, 2, 128, V).transpose(0, 3, 1, 2, 4))

    key = np.asarray(key, dtype=f32)
    value = np.asarray(value, dtype=f32)
    hidden = np.asarray(hidden, dtype=f32)

    wqT = np.ascontiguousarray(np.asarray(Wq, dtype=f32).T).astype(NP_BF16)
    wg1T = np.ascontiguousarray(np.asarray(Wg1, dtype=f32).T).astype(NP_BF16)
    wg2T = np.ascontiguousarray(np.asarray(Wg2, dtype=f32).T).astype(NP_BF16)
    woT = np.ascontiguousarray(np.asarray(Wo, dtype=f32).T).astype(NP_BF16)
    bq = np.ascontiguousarray(np.asarray(bq, dtype=f32)).astype(NP_BF16)
    bg1 = np.ascontiguousarray(np.asarray(bg1, dtype=f32)).astype(NP_BF16)
    bg2 = np.ascontiguousarray(np.asarray(bg2, dtype=f32)).astype(NP_BF16)
    bo = np.ascontiguousarray(np.asarray(bo, dtype=f32)).astype(NP_BF16)

    filled_w = np.minimum(fl + 1, S).astype(f32).reshape(B, 1)
    wp_f = wp.astype(f32).reshape(B, 1)

    in_maps = []
    for c in range(NCORES):
        sel = idx[:, c]
        in_maps.append({
            "keysT": np.ascontiguousarray(keysT[sel]),
            "vpack": np.ascontiguousarray(vpack[sel]),
            "key": np.ascontiguousarray(key[sel]),
            "value": np.ascontiguousarray(value[sel]),
            "hidden": np.ascontiguousarray(hidden[sel]),
            "filled_f": np.ascontiguousarray(filled_w[sel]),
            "wp_f": np.ascontiguousarray(wp_f[sel]),
            "kwp": np.ascontiguousarray(kwp[sel]),
            "vwp": np.ascontiguousarray(vwp[sel]),
            "WqT": wqT, "Wg1T": wg1T, "Wg2T": wg2T, "WoT": woT,
            "bq": bq, "bg1": bg1, "bg2": bg2, "bo": bo,
        })
    return in_maps


def run(trace=False, **inputs):
    idx, pv2, limg = _make_plan(inputs["filled"])
    nc = _get_nc(pv2, limg)
    in_maps = _make_in_maps(idx, **inputs)
    res = run_bass_kernel_spmd(nc, in_maps, core_ids=list(range(NCORES)),
                               trace=trace)
    out = np.empty((B, V), np.float32)
    for c in range(NCORES):
        out[idx[:, c]] = res.results[c]["out"]
    return out, res


def kernel(**inputs) -> np.ndarray:
    out, _ = run(trace=False, **inputs)
    return out


# revision 22
# speedup vs baseline: 1.2043x; 1.1743x over previous
"""EpisodicMemory Trainium2 kernel (8 NeuronCores, pure data parallel over batch).

Reference semantics (per batch b):
    keys_w   = keys   with row write_ptr[b] <- key[b]
    values_w = values with row write_ptr[b] <- value[b]
    filled_w = min(filled + 1, S)
    query    = hidden @ Wq.T + bq
    scores   = (keys_w @ query) / sqrt(K), masked to s < filled_w
    attn     = softmax(scores)
    retrieved= attn @ values_w
    g        = silu([hidden|retrieved] @ Wg1.T + bg1)
    gate     = sigmoid(g @ Wg2.T + bg2)
    out      = (hidden + gate*retrieved) @ Wo.T + bo

The scatter is never materialized: base scores/retrieved are computed from the
original keys/values and corrected algebraically with the old rows at
write_ptr (host-gathered) plus the new key/value rows.

v3 design:
  * keys host-transposed to [K, S] bf16 -> scores are PE matmuls (contract K).
    16 batches accumulate into one [16, 512] PSUM bank via one-hot query
    columns (out partition offsets must be 0 mod 32, so rows are selected by
    zero-padding the stationary operand instead).
  * values in fp8e4, host-packed in (s%128, s//256, (s//128)%2, v) order for
    perf_mode=DoubleRow matmuls (2 fp8 rows per PE cell); attention weights
    are scaled by 128 into fp8 range (denominator scales identically so the
    softmax normalization cancels the factor).  One-hot diagonal layout lets
    all 16 batches of a group accumulate into one [16, 512] PSUM bank.
  * rows s >= filled_w never contribute (scores masked to -inf), so slot i
    only reads/computes ceil-rounded row counts baked from the host-sorted
    filled profile: batches sorted by filled_w desc, rank 8i+c -> core c
    slot i, so all 8 cores share one compiled program.
"""

import sys

sys.path.insert(0, "/opt/trn_rl_repo")

import numpy as np
import ml_dtypes

import concourse.bacc as bacc
import concourse.tile as tile
from concourse import bass, mybir
from concourse.bass_utils import run_bass_kernel_spmd
from concourse.masks import make_identity

B, S, K, V = 512, 1024, 128, 512
NCORES = 8
NB = B // NCORES          # 64 batches per core
T2 = S // 256             # 4 value double-chunks of 256 rows
GRP = 16                  # batches per softmax group
NG = NB // GRP            # 4 groups
SCALE = float(np.sqrt(K))
NEG_BIG = -3.0e37
LN_ATT = float(np.log(128.0))   # attn weights scaled x128 into fp8 range

F32 = mybir.dt.float32
BF16 = mybir.dt.bfloat16
F8 = mybir.dt.float8e4
NP_BF16 = np.dtype(ml_dtypes.bfloat16)
NP_F8 = np.dtype(ml_dtypes.float8_e4m3)


def _build(pv2, limg):
    """pv2[i]: value 256-row double-chunks for slot i (1..4); limg[g]: key rows
    (multiple of 8) read/scored for group g.  Slots sorted descending."""
    nc = bacc.Bacc()
    dt = F32
    DR = mybir.MatmulPerfMode.DoubleRow

    keysT_t = nc.dram_tensor("keysT", [NB, K, S], F8, kind="ExternalInput")
    vpack_t = nc.dram_tensor("vpack", [NB, 128, T2, 2, V], F8, kind="ExternalInput")
    key_t = nc.dram_tensor("key", [NB, K], dt, kind="ExternalInput")
    value_t = nc.dram_tensor("value", [NB, V], dt, kind="ExternalInput")
    hidden_t = nc.dram_tensor("hidden", [NB, V], dt, kind="ExternalInput")
    filled_t = nc.dram_tensor("filled_f", [NB, 1], dt, kind="ExternalInput")
    wp_t = nc.dram_tensor("wp_f", [NB, 1], dt, kind="ExternalInput")
    kwp_t = nc.dram_tensor("kwp", [NB, K], dt, kind="ExternalInput")
    vwp_t = nc.dram_tensor("vwp", [NB, V], dt, kind="ExternalInput")
    wqT_t = nc.dram_tensor("WqT", [V, K], BF16, kind="ExternalInput")
    wg1T_t = nc.dram_tensor("Wg1T", [2 * V, V], BF16, kind="ExternalInput")
    wg2T_t = nc.dram_tensor("Wg2T", [V, V], BF16, kind="ExternalInput")
    woT_t = nc.dram_tensor("WoT", [V, V], BF16, kind="ExternalInput")
    bq_t = nc.dram_tensor("bq", [K], BF16, kind="ExternalInput")
    bg1_t = nc.dram_tensor("bg1", [V], BF16, kind="ExternalInput")
    bg2_t = nc.dram_tensor("bg2", [V], BF16, kind="ExternalInput")
    bo_t = nc.dram_tensor("bo", [V], BF16, kind="ExternalInput")
    out_t = nc.dram_tensor("out", [NB, V], dt, kind="ExternalOutput")

    kview = keysT_t[:].rearrange("b k s -> k b s")

    with tile.TileContext(nc) as tc:
        with (
            tc.tile_pool(name="const", bufs=1) as const,
            tc.tile_pool(name="ktile", bufs=5) as ktile_p,
            tc.tile_pool(name="vtile", bufs=5) as vtile_p,
            tc.tile_pool(name="grp", bufs=2) as grp_p,
            tc.tile_pool(name="sm", bufs=1) as sm_p,
            tc.tile_pool(name="misc", bufs=1) as misc,
            tc.tile_pool(name="ps_sc", bufs=2, space="PSUM") as ps_sc,
            tc.tile_pool(name="ps_gw", bufs=2, space="PSUM") as ps_gw,
            tc.tile_pool(name="ps_tr", bufs=2, space="PSUM") as ps_tr,
        ):
            # ---------------- setup ----------------
            hidden_sb = misc.tile([NB, V], dt)
            nc.scalar.dma_start(out=hidden_sb[:], in_=hidden_t[:, :])
            wqT = const.tile([128, 4, K], BF16)
            nc.scalar.dma_start(out=wqT[:], in_=wqT_t[:].rearrange("(c p) k -> p c k", p=128))
            bq_row = const.tile([1, K], BF16)
            nc.scalar.dma_start(out=bq_row[:], in_=bq_t[None, :])

            identity = const.tile([128, 128], dt)
            make_identity(nc, identity[:])
            identity_bf = const.tile([128, 128], BF16)
            nc.vector.tensor_copy(out=identity_bf[:], in_=identity[:])
            ones_row = const.tile([1, 128], dt)
            nc.vector.memset(ones_row[:], 1.0)
            ones_bf = const.tile([1, 128], BF16)
            nc.vector.memset(ones_bf[:], 1.0)

            iota_i = misc.tile([GRP, S], mybir.dt.int16)
            nc.gpsimd.iota(iota_i[:], pattern=[[1, S]], base=0, channel_multiplier=0)
            iota_f = const.tile([GRP, S], dt)
            nc.vector.tensor_copy(out=iota_f[:], in_=iota_i[:])

            key_sb = misc.tile([NB, K], dt)
            nc.scalar.dma_start(out=key_sb[:], in_=key_t[:, :])
            value_sb = misc.tile([NB, V], dt)
            nc.scalar.dma_start(out=value_sb[:], in_=value_t[:, :])
            filled_sb = misc.tile([NB, 1], dt)
            nc.scalar.dma_start(out=filled_sb[:], in_=filled_t[:, :])
            wp_sb = misc.tile([NB, 1], dt)
            nc.scalar.dma_start(out=wp_sb[:], in_=wp_t[:, :])
            kwp_sb = misc.tile([NB, K], dt)
            nc.scalar.dma_start(out=kwp_sb[:], in_=kwp_t[:, :])
            vwp_sb = misc.tile([NB, V], dt)
            nc.scalar.dma_start(out=vwp_sb[:], in_=vwp_t[:, :])

            # hidden in bf16 + hiddenT (128v x 64b) chunks for the matmuls
            hidden_bf = misc.tile([NB, V], BF16)
            nc.vector.tensor_copy(out=hidden_bf[:], in_=hidden_sb[:])
            hT = misc.tile([128, 4, NB], BF16)
            for c in range(4):
                tp = ps_tr.tile([128, NB], BF16, tag="tr")
                nc.tensor.transpose(out=tp[:], in_=hidden_bf[:, c * 128:(c + 1) * 128],
                                    identity=identity_bf[:NB, :NB])
                nc.scalar.copy(out=hT[:, c, :], in_=tp[:])

            # query both ways: qT (128k x 64b) for scores, q (64b x 128k) for
            # the write-row correction dot products
            qT_ps = ps_tr.tile([K, NB], dt, tag="tr")
            for c in range(4):
                nc.tensor.matmul(out=qT_ps[:], lhsT=wqT[:, c, :], rhs=hT[:, c, :],
                                 start=(c == 0), stop=False)
            nc.tensor.matmul(out=qT_ps[:], lhsT=bq_row[:], rhs=ones_bf[:, :NB],
                             start=False, stop=True)
            qT_f8 = misc.tile([K, NB], F8)
            nc.scalar.copy(out=qT_f8[:], in_=qT_ps[:])

            q_ps = ps_tr.tile([NB, K], dt, tag="tr")
            for c in range(4):
                nc.tensor.matmul(out=q_ps[:], lhsT=hT[:, c, :], rhs=wqT[:, c, :],
                                 start=(c == 0), stop=False)
            nc.tensor.matmul(out=q_ps[:], lhsT=ones_bf[:, :NB], rhs=bq_row[:],
                             start=False, stop=True)
            query_sb = misc.tile([NB, K], dt)
            nc.vector.tensor_copy(out=query_sb[:], in_=q_ps[:])

            junk_rd = misc.tile([NB, K], dt)
            sold = misc.tile([NB, 1], dt)
            nc.vector.tensor_mul(out=junk_rd[:], in0=kwp_sb[:], in1=query_sb[:])
            nc.vector.tensor_reduce(out=sold[:], in_=junk_rd[:],
                                    axis=mybir.AxisListType.X, op=mybir.AluOpType.add)
            snew = misc.tile([NB, 1], dt)
            nc.vector.tensor_mul(out=junk_rd[:], in0=key_sb[:], in1=query_sb[:])
            nc.vector.tensor_reduce(out=snew[:], in_=junk_rd[:],
                                    axis=mybir.AxisListType.X, op=mybir.AluOpType.add)

            denom0 = misc.tile([NB, 1], dt)
            neg_m_all = misc.tile([NB, 1], dt)
            attn_groups = []
            g_sb = misc.tile([NB, V], dt)

            def scores_stage(g):
                b0 = g * GRP
                lim = limg[g]
                nA = min(lim, 512)
                nB = max(lim - 512, 0)
                pvmax = pv2[b0]
                tcap = 2 * pvmax

                filled_g = sm_p.tile([GRP, 1], dt, tag="filled_g")
                nc.gpsimd.dma_start(out=filled_g[:], in_=filled_t[b0:b0 + GRP, :])
                penalty_g = sm_p.tile([GRP, S], dt, tag="penalty_g")
                nc.vector.tensor_scalar(
                    out=penalty_g[:], in0=iota_f[:], scalar1=filled_g[:, :1],
                    scalar2=NEG_BIG, op0=mybir.AluOpType.is_ge, op1=mybir.AluOpType.mult)

                # keysT sub-DMAs (2 slots/transfer for group 0 to cut startup)
                ksub = 2 if g == 0 else 4
                kts = []
                for j in range(GRP // ksub):
                    kt = ktile_p.tile([K, 4, S], F8, tag="ktile")
                    nc.gpsimd.dma_start(
                        out=kt[:, :ksub, :lim],
                        in_=kview[:, b0 + ksub * j:b0 + ksub * (j + 1), :lim])
                    kts.append(kt)

                # one-hot query columns: qoh[:, m, c] = qT[:, b0+c] iff m == c
                qoh = grp_p.tile([K, GRP, GRP], F8, tag="qoh")
                nc.vector.memset(qoh[:], 0.0)
                qa = qoh[:, :, :]
                qdiag = bass.AP(tensor=qa.tensor, offset=qa.offset,
                                ap=[qa.ap[0], [GRP + 1, GRP]])
                nc.scalar.copy(out=qdiag, in_=qT_f8[:, b0:b0 + GRP])

                # scores: 16 accumulating fp8 matmuls per 512-col bank
                sc_a = ps_sc.tile([GRP, 512], dt, tag="sc_a")
                for bl in range(GRP):
                    nc.tensor.matmul(out=sc_a[:, :nA], lhsT=qoh[:, bl, :],
                                     rhs=kts[bl // ksub][:, bl % ksub, :nA],
                                     start=(bl == 0), stop=(bl == GRP - 1))
                if nB:
                    sc_b = ps_sc.tile([GRP, 512], dt, tag="sc_b")
                    for bl in range(GRP):
                        nc.tensor.matmul(out=sc_b[:, :nB], lhsT=qoh[:, bl, :],
                                         rhs=kts[bl // ksub][:, bl % ksub, 512:512 + nB],
                                         start=(bl == 0), stop=(bl == GRP - 1))

                # scores + penalty -> SBUF rows; tail past lim is pure penalty
                scores_g = sm_p.tile([GRP, S], dt, tag="scores_g")
                nc.vector.tensor_tensor(out=scores_g[:, :nA], in0=sc_a[:, :nA],
                                        in1=penalty_g[:, :nA], op=mybir.AluOpType.add)
                if nB:
                    nc.vector.tensor_tensor(out=scores_g[:, 512:512 + nB],
                                            in0=sc_b[:, :nB],
                                            in1=penalty_g[:, 512:512 + nB],
                                            op=mybir.AluOpType.add)
                if lim < S:
                    nc.vector.tensor_copy(out=scores_g[:, lim:],
                                          in_=penalty_g[:, lim:])

                m_g = sm_p.tile([GRP, 1], dt, tag="m_g")
                nc.vector.tensor_reduce(out=m_g[:], in_=scores_g[:],
                                        axis=mybir.AxisListType.X,
                                        op=mybir.AluOpType.max)
                # bias = -m/SCALE + ln(128): scales attn x128 into fp8 range
                neg_m_g = sm_p.tile([GRP, 1], dt, tag="neg_m_g")
                nc.vector.tensor_scalar(
                    out=neg_m_g[:], in0=m_g[:], scalar1=-1.0 / SCALE,
                    scalar2=LN_ATT, op0=mybir.AluOpType.mult, op1=mybir.AluOpType.add)
                exps_g = sm_p.tile([GRP, S], dt, tag="exps_g")
                denom0_g = sm_p.tile([GRP, 1], dt, tag="denom0_g")
                nc.scalar.activation(
                    out=exps_g[:], in_=scores_g[:],
                    func=mybir.ActivationFunctionType.Exp,
                    bias=neg_m_g[:, :1], scale=1.0 / SCALE,
                    accum_out=denom0_g[:, :1])

                # one-hot diagonal attn in fp8: aoh[:, t2, i, m, c] nonzero only
                # at m == c (DoubleRow lhsT [128, 2, 16] slices at fixed c)
                aoh = grp_p.tile([128, T2, 2, GRP, GRP], F8, tag="aoh")
                nc.vector.memset(aoh[:, :pvmax], 0.0)
                exps_v = exps_g[:].rearrange("g (t x) -> g t x", x=128)
                for t in range(tcap):
                    tp = ps_tr.tile([128, GRP], dt, tag="tr")
                    nc.tensor.transpose(out=tp[:], in_=exps_v[:, t, :],
                                        identity=identity[:GRP, :GRP])
                    da = aoh[:, t // 2, t % 2, :, :]
                    diag = bass.AP(tensor=da.tensor, offset=da.offset,
                                   ap=[da.ap[0], [GRP + 1, GRP]])
                    nc.scalar.copy(out=diag, in_=tp[:])
                attn_groups.append(aoh)

                nc.gpsimd.dma_start(out=denom0[b0:b0 + GRP, :], in_=denom0_g[:])
                nc.gpsimd.dma_start(out=neg_m_all[b0:b0 + GRP, :], in_=neg_m_g[:])

            def values_stage(g):
                b0 = g * GRP
                aoh = attn_groups[g]
                vts = []
                for j in range(4):
                    pm = pv2[b0 + 4 * j]     # subgroup max (sorted desc)
                    vt = vtile_p.tile([128, 4, T2, 2, V], F8, tag="vtile")
                    nc.sync.dma_start(out=vt[:, :, :pm], in_=vpack_t[b0 + 4 * j:b0 + 4 * j + 4]
                                      .rearrange("b p t i v -> p b t i v")[:, :, :pm])
                    vts.append(vt)
                steps = [(bl, t2) for bl in range(GRP) for t2 in range(pv2[b0 + bl])]
                gw = ps_gw.tile([GRP, V], F32, tag="gw")
                for si, (bl, t2) in enumerate(steps):
                    nc.tensor.matmul(out=gw[:], lhsT=aoh[:, t2, :, :, bl],
                                     rhs=vts[bl // 4][:, bl % 4, t2, :, :],
                                     start=(si == 0), stop=(si == len(steps) - 1),
                                     perf_mode=mybir.MatmulPerfMode.DoubleRow)
                gtmp = grp_p.tile([GRP, V], dt, tag="gtmp")
                nc.scalar.copy(out=gtmp[:], in_=gw[:])
                nc.gpsimd.dma_start(out=g_sb[b0:b0 + GRP, :], in_=gtmp[:])

            # persistent tiles shared by the two tail halves
            wg1T = const.tile([128, 8, V], BF16)
            wg2T = const.tile([128, 4, V], BF16)
            woT = const.tile([128, 4, V], BF16)
            bg1_row = const.tile([1, V], BF16)
            bg2_row = const.tile([1, V], BF16)
            bo_row = const.tile([1, V], BF16)
            eo = misc.tile([NB, 1], dt)
            en = misc.tile([NB, 1], dt)
            mask_wp = misc.tile([NB, 1], dt)
            a_old = misc.tile([NB, 1], dt)
            a_new = misc.tile([NB, 1], dt)
            denom = misc.tile([NB, 1], dt)
            recip = misc.tile([NB, 1], dt)
            t1 = misc.tile([NB, V], dt)
            t2_ = misc.tile([NB, V], dt)
            retr = misc.tile([NB, V], dt)
            retr_bf = misc.tile([NB, V], BF16)
            rT = misc.tile([128, 4, NB], BF16)
            g_act = misc.tile([NB, V], dt)
            g_act_bf = misc.tile([NB, V], BF16)
            gT = misc.tile([128, 4, NB], BF16)
            gate = misc.tile([NB, V], dt)
            z = misc.tile([NB, V], dt)
            z_bf = misc.tile([NB, V], BF16)
            zT = misc.tile([128, 4, NB], BF16)
            out_sb = misc.tile([NB, V], dt)

            def load_mlp_weights():
                nc.scalar.dma_start(out=wg1T[:], in_=wg1T_t[:].rearrange("(c p) j -> p c j", p=128))
                nc.scalar.dma_start(out=wg2T[:], in_=wg2T_t[:].rearrange("(c p) j -> p c j", p=128))
                nc.scalar.dma_start(out=woT[:], in_=woT_t[:].rearrange("(c p) j -> p c j", p=128))
                nc.scalar.dma_start(out=bg1_row[:], in_=bg1_t[None, :])
                nc.scalar.dma_start(out=bg2_row[:], in_=bg2_t[None, :])
                nc.scalar.dma_start(out=bo_row[:], in_=bo_t[None, :])

            def exps_half(h0, H):
                """write-row correction exps for batches [h0, h0+H)."""
                hs = slice(h0, h0 + H)
                nc.scalar.activation(out=eo[hs], in_=sold[hs],
                                     func=mybir.ActivationFunctionType.Exp,
                                     bias=neg_m_all[hs, :1], scale=1.0 / SCALE)
                nc.scalar.activation(out=en[hs], in_=snew[hs],
                                     func=mybir.ActivationFunctionType.Exp,
                                     bias=neg_m_all[hs, :1], scale=1.0 / SCALE)

            def corr_half(h0, H):
                """denominator + retrieved for batches [h0, h0+H) (DVE)."""
                hs = slice(h0, h0 + H)
                nc.vector.tensor_tensor(out=mask_wp[hs], in0=wp_sb[hs],
                                        in1=filled_sb[hs], op=mybir.AluOpType.is_lt)
                nc.vector.tensor_mul(out=a_old[hs], in0=eo[hs], in1=mask_wp[hs])
                nc.vector.tensor_mul(out=a_new[hs], in0=en[hs], in1=mask_wp[hs])
                nc.vector.tensor_sub(out=denom[hs], in0=denom0[hs], in1=a_old[hs])
                nc.vector.tensor_add(out=denom[hs], in0=denom[hs], in1=a_new[hs])
                nc.vector.reciprocal(out=recip[hs], in_=denom[hs])

                nc.vector.tensor_scalar_mul(out=t1[hs], in0=value_sb[hs],
                                            scalar1=a_new[hs, :1])
                nc.vector.tensor_scalar_mul(out=t2_[hs], in0=vwp_sb[hs],
                                            scalar1=a_old[hs, :1])
                nc.vector.tensor_sub(out=t1[hs], in0=t1[hs], in1=t2_[hs])
                nc.vector.tensor_add(out=t1[hs], in0=g_sb[hs], in1=t1[hs])
                nc.vector.tensor_scalar_mul(out=retr[hs], in0=t1[hs],
                                            scalar1=recip[hs, :1])
                nc.vector.tensor_copy(out=retr_bf[hs], in_=retr[hs])

            def mlp_half(h0, H):
                """gated MLP for batches [h0, h0+H) (PE-heavy)."""
                hs = slice(h0, h0 + H)
                idb = identity_bf[hs, hs]   # 32x32 identity block at base h0
                for c in range(4):
                    tp = ps_tr.tile([128, H], BF16, tag="tr")
                    nc.tensor.transpose(out=tp[:], in_=retr_bf[hs, c * 128:(c + 1) * 128],
                                        identity=idb)
                    nc.scalar.copy(out=rT[:, c, h0:h0 + H], in_=tp[:])

                g_ps = ps_tr.tile([NB, V], dt, tag="tr")
                for ic in range(8):
                    lhsT = hT[:, ic, h0:h0 + H] if ic < 4 else rT[:, ic - 4, h0:h0 + H]
                    nc.tensor.matmul(out=g_ps[hs, :], lhsT=lhsT, rhs=wg1T[:, ic, :],
                                     start=(ic == 0), stop=False)
                nc.tensor.matmul(out=g_ps[hs, :], lhsT=ones_bf[:, :H], rhs=bg1_row[:],
                                 start=False, stop=True)
                nc.scalar.activation(out=g_act[hs], in_=g_ps[hs, :],
                                     func=mybir.ActivationFunctionType.Sigmoid)
                nc.vector.tensor_mul(out=g_act[hs], in0=g_act[hs], in1=g_ps[hs, :])

                nc.vector.tensor_copy(out=g_act_bf[hs], in_=g_act[hs])
                for c in range(4):
                    tp = ps_tr.tile([128, H], BF16, tag="tr")
                    nc.tensor.transpose(out=tp[:], in_=g_act_bf[hs, c * 128:(c + 1) * 128],
                                        identity=idb)
                    nc.scalar.copy(out=gT[:, c, h0:h0 + H], in_=tp[:])

                gate_ps = ps_tr.tile([NB, V], dt, tag="tr")
                for c in range(4):
                    nc.tensor.matmul(out=gate_ps[hs, :], lhsT=gT[:, c, h0:h0 + H],
                                     rhs=wg2T[:, c, :], start=(c == 0), stop=False)
                nc.tensor.matmul(out=gate_ps[hs, :], lhsT=ones_bf[:, :H],
                                 rhs=bg2_row[:], start=False, stop=True)
                nc.scalar.activation(out=gate[hs], in_=gate_ps[hs, :],
                                     func=mybir.ActivationFunctionType.Sigmoid)

                nc.vector.tensor_mul(out=z[hs], in0=gate[hs], in1=retr[hs])
                nc.vector.tensor_add(out=z[hs], in0=z[hs], in1=hidden_sb[hs])
                nc.vector.tensor_copy(out=z_bf[hs], in_=z[hs])
                for c in range(4):
                    tp = ps_tr.tile([128, H], BF16, tag="tr")
                    nc.tensor.transpose(out=tp[:], in_=z_bf[hs, c * 128:(c + 1) * 128],
                                        identity=idb)
                    nc.scalar.copy(out=zT[:, c, h0:h0 + H], in_=tp[:])

                out_ps = ps_tr.tile([NB, V], dt, tag="tr")
                for c in range(4):
                    nc.tensor.matmul(out=out_ps[hs, :], lhsT=zT[:, c, h0:h0 + H],
                                     rhs=woT[:, c, :], start=(c == 0), stop=False)
                nc.tensor.matmul(out=out_ps[hs, :], lhsT=ones_bf[:, :H],
                                 rhs=bo_row[:], start=False, stop=True)
                nc.vector.tensor_copy(out=out_sb[hs], in_=out_ps[hs, :])
                nc.sync.dma_start(out=out_t[hs, :], in_=out_sb[hs])

            scores_stage(0)
            values_stage(0)
            load_mlp_weights()
            scores_stage(1)
            values_stage(1)
            scores_stage(2)
            values_stage(2)
            scores_stage(3)
            exps_half(0, NB)    # needs all neg_m stitches (after scores(3))
            values_stage(3)
            corr_half(0, NB)
            mlp_half(0, NB)

    nc.finalize()
    return nc


_NC_CACHE = {}


def _get_nc(pv2, limg):
    key = (tuple(pv2), tuple(limg))
    if key not in _NC_CACHE:
        _NC_CACHE.clear()
        _NC_CACHE[key] = _build(tuple(pv2), tuple(limg))
    return _NC_CACHE[key]


def _make_plan(filled):
    fl = np.asarray(filled).astype(np.int64)
    f_w = np.minimum(fl + 1, S)
    order = np.argsort(-f_w, kind="stable")
    idx = order.reshape(NB, NCORES)          # slot i, core c -> batch idx[i, c]
    fmax = f_w[idx[:, 0]]
    pv2 = np.minimum((fmax + 255) // 256, T2).astype(np.int64)
    limg = tuple(int(min((fmax[g * GRP] + 7) // 8 * 8, S)) for g in range(NG))
    return idx, tuple(int(x) for x in pv2), limg


def _make_in_maps(idx, keys, values, key, value, hidden, write_ptr, filled,
                  Wq, bq, Wg1, bg1, Wg2, bg2, Wo, bo):
    f32 = np.float32
    bidx = np.arange(B)
    wp = np.asarray(write_ptr).astype(np.int64)
    fl = np.asarray(filled).astype(np.int64)

    keys_f8 = np.asarray(keys, dtype=f32).astype(NP_F8)
    kwp = keys_f8[bidx, wp].astype(f32)
    keysT = np.ascontiguousarray(keys_f8.transpose(0, 2, 1))      # [B, K, S]

    values_f8 = np.asarray(values, dtype=f32).astype(NP_F8)
    vwp = values_f8[bidx, wp].astype(f32)
    # vpack[b, p, t2, i, v] = values[b, t2*256 + i*128 + p, v]
    vpack = np.ascontiguousarray(
        values_f8.reshape(B, T2, 2, 128, V).transpose(0, 3, 1, 2, 4))

    key = np.asarray(key, dtype=f32)
    value = np.asarray(value, dtype=f32)
    hidden = np.asarray(hidden, dtype=f32)

    wqT = np.ascontiguousarray(np.asarray(Wq, dtype=f32).T).astype(NP_BF16)
    wg1T = np.ascontiguousarray(np.asarray(Wg1, dtype=f32).T).astype(NP_BF16)
    wg2T = np.ascontiguousarray(np.asarray(Wg2, dtype=f32).T).astype(NP_BF16)
    woT = np.ascontiguousarray(np.asarray(Wo, dtype=f32).T).astype(NP_BF16)
    bq = np.ascontiguousarray(np.asarray(bq, dtype=f32)).astype(NP_BF16)
    bg1 = np.ascontiguousarray(np.asarray(bg1, dtype=f32)).astype(NP_BF16)
    bg2 = np.ascontiguousarray(np.asarray(bg2, dtype=f32)).astype(NP_BF16)
    bo = np.ascontiguousarray(np.asarray(bo, dtype=f32)).astype(NP_BF16)

    filled_w = np.minimum(fl + 1, S).astype(f32).reshape(B, 1)
    wp_f = wp.astype(f32).reshape(B, 1)

    in_maps = []
    for c in range(NCORES):
        sel = idx[:, c]
        in_maps.append({
            "keysT": np.ascontiguousarray(keysT[sel]),
            "vpack": np.ascontiguousarray(vpack[sel]),
            "key": np.ascontiguousarray(key[sel]),
            "value": np.ascontiguousarray(value[sel]),
            "hidden": np.ascontiguousarray(hidden[sel]),
            "filled_f": np.ascontiguousarray(filled_w[sel]),
            "wp_f": np.ascontiguousarray(wp_f[sel]),
            "kwp": np.ascontiguousarray(kwp[sel]),
            "vwp": np.ascontiguousarray(vwp[sel]),
            "WqT": wqT, "Wg1T": wg1T, "Wg2T": wg2T, "WoT": woT,
            "bq": bq, "bg1": bg1, "bg2": bg2, "bo": bo,
        })
    return in_maps


def run(trace=False, **inputs):
    idx, pv2, limg = _make_plan(inputs["filled"])
    nc = _get_nc(pv2, limg)
    in_maps = _make_in_maps(idx, **inputs)
    res = run_bass_kernel_spmd(nc, in_maps, core_ids=list(range(NCORES)),
                               trace=trace)
    out = np.empty((B, V), np.float32)
    for c in range(NCORES):
        out[idx[:, c]] = res.results[c]["out"]
    return out, res


def kernel(**inputs) -> np.ndarray:
    out, _ = run(trace=False, **inputs)
    return out


# revision 23
# speedup vs baseline: 1.2193x; 1.0124x over previous
"""EpisodicMemory Trainium2 kernel (8 NeuronCores, pure data parallel over batch).

Reference semantics (per batch b):
    keys_w   = keys   with row write_ptr[b] <- key[b]
    values_w = values with row write_ptr[b] <- value[b]
    filled_w = min(filled + 1, S)
    query    = hidden @ Wq.T + bq
    scores   = (keys_w @ query) / sqrt(K), masked to s < filled_w
    attn     = softmax(scores)
    retrieved= attn @ values_w
    g        = silu([hidden|retrieved] @ Wg1.T + bg1)
    gate     = sigmoid(g @ Wg2.T + bg2)
    out      = (hidden + gate*retrieved) @ Wo.T + bo

The scatter is never materialized: base scores/retrieved are computed from the
original keys/values and corrected algebraically with the old rows at
write_ptr (host-gathered) plus the new key/value rows.

v3 design:
  * keys host-transposed to [K, S] bf16 -> scores are PE matmuls (contract K).
    16 batches accumulate into one [16, 512] PSUM bank via one-hot query
    columns (out partition offsets must be 0 mod 32, so rows are selected by
    zero-padding the stationary operand instead).
  * values in fp8e4, host-packed in (s%128, s//256, (s//128)%2, v) order for
    perf_mode=DoubleRow matmuls (2 fp8 rows per PE cell); attention weights
    are scaled by 128 into fp8 range (denominator scales identically so the
    softmax normalization cancels the factor).  One-hot diagonal layout lets
    all 16 batches of a group accumulate into one [16, 512] PSUM bank.
  * rows s >= filled_w never contribute (scores masked to -inf), so slot i
    only reads/computes ceil-rounded row counts baked from the host-sorted
    filled profile: batches sorted by filled_w desc, rank 8i+c -> core c
    slot i, so all 8 cores share one compiled program.
"""

import sys

sys.path.insert(0, "/opt/trn_rl_repo")

import numpy as np
import ml_dtypes

import concourse.bacc as bacc
import concourse.tile as tile
from concourse import bass, mybir
from concourse.bass_utils import run_bass_kernel_spmd
from concourse.masks import make_identity

B, S, K, V = 512, 1024, 128, 512
NCORES = 8
NB = B // NCORES          # 64 batches per core
T2 = S // 256             # 4 value double-chunks of 256 rows
GRP = 16                  # batches per softmax group
NG = NB // GRP            # 4 groups
SCALE = float(np.sqrt(K))
NEG_BIG = -3.0e37
LN_ATT = float(np.log(128.0))   # attn weights scaled x128 into fp8 range

F32 = mybir.dt.float32
BF16 = mybir.dt.bfloat16
F8 = mybir.dt.float8e4
NP_BF16 = np.dtype(ml_dtypes.bfloat16)
NP_F8 = np.dtype(ml_dtypes.float8_e4m3)


def _build(pv2, limg):
    """pv2[i]: value 256-row double-chunks for slot i (1..4); limg[g]: key rows
    (multiple of 8) read/scored for group g.  Slots sorted descending."""
    nc = bacc.Bacc()
    dt = F32
    DR = mybir.MatmulPerfMode.DoubleRow

    keysT_t = nc.dram_tensor("keysT", [NB, K, S], F8, kind="ExternalInput")
    vpack_t = nc.dram_tensor("vpack", [NB, 128, T2, 2, V], F8, kind="ExternalInput")
    key_t = nc.dram_tensor("key", [NB, K], dt, kind="ExternalInput")
    value_t = nc.dram_tensor("value", [NB, V], dt, kind="ExternalInput")
    hidden_t = nc.dram_tensor("hidden", [NB, V], dt, kind="ExternalInput")
    filled_t = nc.dram_tensor("filled_f", [NB, 1], dt, kind="ExternalInput")
    wp_t = nc.dram_tensor("wp_f", [NB, 1], dt, kind="ExternalInput")
    kwp_t = nc.dram_tensor("kwp", [NB, K], dt, kind="ExternalInput")
    vwp_t = nc.dram_tensor("vwp", [NB, V], dt, kind="ExternalInput")
    wqT_t = nc.dram_tensor("WqT", [V, K], BF16, kind="ExternalInput")
    wg1T_t = nc.dram_tensor("Wg1T", [2 * V, V], BF16, kind="ExternalInput")
    wg2T_t = nc.dram_tensor("Wg2T", [V, V], BF16, kind="ExternalInput")
    woT_t = nc.dram_tensor("WoT", [V, V], BF16, kind="ExternalInput")
    bq_t = nc.dram_tensor("bq", [K], BF16, kind="ExternalInput")
    bg1_t = nc.dram_tensor("bg1", [V], BF16, kind="ExternalInput")
    bg2_t = nc.dram_tensor("bg2", [V], BF16, kind="ExternalInput")
    bo_t = nc.dram_tensor("bo", [V], BF16, kind="ExternalInput")
    out_t = nc.dram_tensor("out", [NB, V], dt, kind="ExternalOutput")

    kview = keysT_t[:].rearrange("b k s -> k b s")

    with tile.TileContext(nc) as tc:
        with (
            tc.tile_pool(name="const", bufs=1) as const,
            tc.tile_pool(name="ktile", bufs=5) as ktile_p,
            tc.tile_pool(name="vtile", bufs=5) as vtile_p,
            tc.tile_pool(name="grp", bufs=2) as grp_p,
            tc.tile_pool(name="sm", bufs=1) as sm_p,
            tc.tile_pool(name="misc", bufs=1) as misc,
            tc.tile_pool(name="ps_sc", bufs=2, space="PSUM") as ps_sc,
            tc.tile_pool(name="ps_gw", bufs=2, space="PSUM") as ps_gw,
            tc.tile_pool(name="ps_tr", bufs=2, space="PSUM") as ps_tr,
        ):
            # ---------------- setup ----------------
            hidden_sb = misc.tile([NB, V], dt)
            nc.scalar.dma_start(out=hidden_sb[:], in_=hidden_t[:, :])
            wqT = const.tile([128, 4, K], BF16)
            nc.scalar.dma_start(out=wqT[:], in_=wqT_t[:].rearrange("(c p) k -> p c k", p=128))
            bq_row = const.tile([1, K], BF16)
            nc.scalar.dma_start(out=bq_row[:], in_=bq_t[None, :])

            identity = const.tile([128, 128], dt)
            make_identity(nc, identity[:])
            identity_bf = const.tile([128, 128], BF16)
            nc.vector.tensor_copy(out=identity_bf[:], in_=identity[:])
            ones_row = const.tile([1, 128], dt)
            nc.vector.memset(ones_row[:], 1.0)
            ones_bf = const.tile([1, 128], BF16)
            nc.vector.memset(ones_bf[:], 1.0)

            iota_i = misc.tile([GRP, S], mybir.dt.int16)
            nc.gpsimd.iota(iota_i[:], pattern=[[1, S]], base=0, channel_multiplier=0)
            iota_f = const.tile([GRP, S], dt)
            nc.vector.tensor_copy(out=iota_f[:], in_=iota_i[:])

            key_sb = misc.tile([NB, K], dt)
            nc.scalar.dma_start(out=key_sb[:], in_=key_t[:, :])
            value_sb = misc.tile([NB, V], dt)
            nc.scalar.dma_start(out=value_sb[:], in_=value_t[:, :])
            filled_sb = misc.tile([NB, 1], dt)
            nc.scalar.dma_start(out=filled_sb[:], in_=filled_t[:, :])
            wp_sb = misc.tile([NB, 1], dt)
            nc.scalar.dma_start(out=wp_sb[:], in_=wp_t[:, :])
            kwp_sb = misc.tile([NB, K], dt)
            nc.scalar.dma_start(out=kwp_sb[:], in_=kwp_t[:, :])
            vwp_sb = misc.tile([NB, V], dt)
            nc.scalar.dma_start(out=vwp_sb[:], in_=vwp_t[:, :])

            # hidden in bf16 + hiddenT (128v x 64b) chunks for the matmuls
            hidden_bf = misc.tile([NB, V], BF16)
            nc.vector.tensor_copy(out=hidden_bf[:], in_=hidden_sb[:])
            hT = misc.tile([128, 4, NB], BF16)
            for c in range(4):
                tp = ps_tr.tile([128, NB], BF16, tag="tr")
                nc.tensor.transpose(out=tp[:], in_=hidden_bf[:, c * 128:(c + 1) * 128],
                                    identity=identity_bf[:NB, :NB])
                nc.scalar.copy(out=hT[:, c, :], in_=tp[:])

            # query both ways: qT (128k x 64b) for scores, q (64b x 128k) for
            # the write-row correction dot products
            qT_ps = ps_tr.tile([K, NB], dt, tag="tr")
            for c in range(4):
                nc.tensor.matmul(out=qT_ps[:], lhsT=wqT[:, c, :], rhs=hT[:, c, :],
                                 start=(c == 0), stop=False)
            nc.tensor.matmul(out=qT_ps[:], lhsT=bq_row[:], rhs=ones_bf[:, :NB],
                             start=False, stop=True)
            qT_f8 = misc.tile([K, NB], F8)
            nc.scalar.copy(out=qT_f8[:], in_=qT_ps[:])

            q_ps = ps_tr.tile([NB, K], dt, tag="tr")
            for c in range(4):
                nc.tensor.matmul(out=q_ps[:], lhsT=hT[:, c, :], rhs=wqT[:, c, :],
                                 start=(c == 0), stop=False)
            nc.tensor.matmul(out=q_ps[:], lhsT=ones_bf[:, :NB], rhs=bq_row[:],
                             start=False, stop=True)
            query_sb = misc.tile([NB, K], dt)
            nc.vector.tensor_copy(out=query_sb[:], in_=q_ps[:])

            junk_rd = misc.tile([NB, K], dt)
            sold = misc.tile([NB, 1], dt)
            nc.vector.tensor_mul(out=junk_rd[:], in0=kwp_sb[:], in1=query_sb[:])
            nc.vector.tensor_reduce(out=sold[:], in_=junk_rd[:],
                                    axis=mybir.AxisListType.X, op=mybir.AluOpType.add)
            snew = misc.tile([NB, 1], dt)
            nc.vector.tensor_mul(out=junk_rd[:], in0=key_sb[:], in1=query_sb[:])
            nc.vector.tensor_reduce(out=snew[:], in_=junk_rd[:],
                                    axis=mybir.AxisListType.X, op=mybir.AluOpType.add)

            denom0 = misc.tile([NB, 1], dt)
            neg_m_all = misc.tile([NB, 1], dt)
            attn_groups = []
            g_sb = misc.tile([NB, V], dt)

            def scores_stage(g):
                b0 = g * GRP
                lim = limg[g]
                nA = min(lim, 512)
                nB = max(lim - 512, 0)
                pvmax = pv2[b0]
                tcap = 2 * pvmax

                filled_g = sm_p.tile([GRP, 1], dt, tag="filled_g")
                nc.gpsimd.dma_start(out=filled_g[:], in_=filled_t[b0:b0 + GRP, :])
                penalty_g = sm_p.tile([GRP, S], dt, tag="penalty_g")
                nc.vector.tensor_scalar(
                    out=penalty_g[:], in0=iota_f[:], scalar1=filled_g[:, :1],
                    scalar2=NEG_BIG, op0=mybir.AluOpType.is_ge, op1=mybir.AluOpType.mult)

                # keysT sub-DMAs (2 slots/transfer for group 0 to cut startup)
                ksub = 2 if g == 0 else 4
                kts = []
                for j in range(GRP // ksub):
                    kt = ktile_p.tile([K, 4, S], F8, tag="ktile")
                    nc.gpsimd.dma_start(
                        out=kt[:, :ksub, :lim],
                        in_=kview[:, b0 + ksub * j:b0 + ksub * (j + 1), :lim])
                    kts.append(kt)

                # one-hot query columns: qoh[:, m, c] = qT[:, b0+c] iff m == c
                qoh = grp_p.tile([K, GRP, GRP], F8, tag="qoh")
                nc.vector.memset(qoh[:], 0.0)
                qa = qoh[:, :, :]
                qdiag = bass.AP(tensor=qa.tensor, offset=qa.offset,
                                ap=[qa.ap[0], [GRP + 1, GRP]])
                nc.scalar.copy(out=qdiag, in_=qT_f8[:, b0:b0 + GRP])

                # scores: 16 accumulating fp8 matmuls per 512-col bank
                sc_a = ps_sc.tile([GRP, 512], dt, tag="sc_a")
                for bl in range(GRP):
                    nc.tensor.matmul(out=sc_a[:, :nA], lhsT=qoh[:, bl, :],
                                     rhs=kts[bl // ksub][:, bl % ksub, :nA],
                                     start=(bl == 0), stop=(bl == GRP - 1))
                if nB:
                    sc_b = ps_sc.tile([GRP, 512], dt, tag="sc_b")
                    for bl in range(GRP):
                        nc.tensor.matmul(out=sc_b[:, :nB], lhsT=qoh[:, bl, :],
                                         rhs=kts[bl // ksub][:, bl % ksub, 512:512 + nB],
                                         start=(bl == 0), stop=(bl == GRP - 1))

                # scores + penalty -> SBUF rows; tail past lim is pure penalty
                scores_g = sm_p.tile([GRP, S], dt, tag="scores_g")
                nc.vector.tensor_tensor(out=scores_g[:, :nA], in0=sc_a[:, :nA],
                                        in1=penalty_g[:, :nA], op=mybir.AluOpType.add)
                if nB:
                    nc.vector.tensor_tensor(out=scores_g[:, 512:512 + nB],
                                            in0=sc_b[:, :nB],
                                            in1=penalty_g[:, 512:512 + nB],
                                            op=mybir.AluOpType.add)
                if lim < S:
                    nc.vector.tensor_copy(out=scores_g[:, lim:],
                                          in_=penalty_g[:, lim:])

                m_g = sm_p.tile([GRP, 1], dt, tag="m_g")
                nc.vector.tensor_reduce(out=m_g[:], in_=scores_g[:],
                                        axis=mybir.AxisListType.X,
                                        op=mybir.AluOpType.max)
                # bias = -m/SCALE + ln(128): scales attn x128 into fp8 range
                neg_m_g = sm_p.tile([GRP, 1], dt, tag="neg_m_g")
                nc.vector.tensor_scalar(
                    out=neg_m_g[:], in0=m_g[:], scalar1=-1.0 / SCALE,
                    scalar2=LN_ATT, op0=mybir.AluOpType.mult, op1=mybir.AluOpType.add)
                exps_g = sm_p.tile([GRP, S], dt, tag="exps_g")
                denom0_g = sm_p.tile([GRP, 1], dt, tag="denom0_g")
                nc.scalar.activation(
                    out=exps_g[:], in_=scores_g[:],
                    func=mybir.ActivationFunctionType.Exp,
                    bias=neg_m_g[:, :1], scale=1.0 / SCALE,
                    accum_out=denom0_g[:, :1])

                # one-hot diagonal attn in fp8: aoh[:, t2, i, m, c] nonzero only
                # at m == c (DoubleRow lhsT [128, 2, 16] slices at fixed c)
                aoh = grp_p.tile([128, T2, 2, GRP, GRP], F8, tag="aoh")
                nc.vector.memset(aoh[:, :pvmax], 0.0)
                exps_v = exps_g[:].rearrange("g (t x) -> g t x", x=128)
                for t in range(tcap):
                    tp = ps_tr.tile([128, GRP], dt, tag="tr")
                    nc.tensor.transpose(out=tp[:], in_=exps_v[:, t, :],
                                        identity=identity[:GRP, :GRP])
                    da = aoh[:, t // 2, t % 2, :, :]
                    diag = bass.AP(tensor=da.tensor, offset=da.offset,
                                   ap=[da.ap[0], [GRP + 1, GRP]])
                    nc.scalar.copy(out=diag, in_=tp[:])
                attn_groups.append(aoh)

                nc.gpsimd.dma_start(out=denom0[b0:b0 + GRP, :], in_=denom0_g[:])
                nc.gpsimd.dma_start(out=neg_m_all[b0:b0 + GRP, :], in_=neg_m_g[:])

            def values_stage(g):
                b0 = g * GRP
                aoh = attn_groups[g]
                vts = []
                for j in range(4):
                    pm = pv2[b0 + 4 * j]     # subgroup max (sorted desc)
                    vt = vtile_p.tile([128, 4, T2, 2, V], F8, tag="vtile")
                    nc.sync.dma_start(out=vt[:, :, :pm], in_=vpack_t[b0 + 4 * j:b0 + 4 * j + 4]
                                      .rearrange("b p t i v -> p b t i v")[:, :, :pm])
                    vts.append(vt)
                steps = [(bl, t2) for bl in range(GRP) for t2 in range(pv2[b0 + bl])]
                gw = ps_gw.tile([GRP, V], F32, tag="gw")
                for si, (bl, t2) in enumerate(steps):
                    nc.tensor.matmul(out=gw[:], lhsT=aoh[:, t2, :, :, bl],
                                     rhs=vts[bl // 4][:, bl % 4, t2, :, :],
                                     start=(si == 0), stop=(si == len(steps) - 1),
                                     perf_mode=mybir.MatmulPerfMode.DoubleRow)
                gtmp = grp_p.tile([GRP, V], dt, tag="gtmp")
                nc.scalar.copy(out=gtmp[:], in_=gw[:])
                nc.gpsimd.dma_start(out=g_sb[b0:b0 + GRP, :], in_=gtmp[:])

            # persistent tiles shared by the two tail halves
            wg1T = const.tile([128, 8, V], BF16)
            wg2T = const.tile([128, 4, V], BF16)
            woT = const.tile([128, 4, V], BF16)
            bg1_row = const.tile([1, V], BF16)
            bg2_row = const.tile([1, V], BF16)
            bo_row = const.tile([1, V], BF16)
            eo = misc.tile([NB, 1], dt)
            en = misc.tile([NB, 1], dt)
            mask_wp = misc.tile([NB, 1], dt)
            a_old = misc.tile([NB, 1], dt)
            a_new = misc.tile([NB, 1], dt)
            denom = misc.tile([NB, 1], dt)
            recip = misc.tile([NB, 1], dt)
            t1 = misc.tile([NB, V], dt)
            t2_ = misc.tile([NB, V], dt)
            retr = misc.tile([NB, V], dt)
            retr_bf = misc.tile([NB, V], BF16)
            rT = misc.tile([128, 4, NB], BF16)
            g_act = misc.tile([NB, V], dt)
            g_act_bf = misc.tile([NB, V], BF16)
            gT = misc.tile([128, 4, NB], BF16)
            gate = misc.tile([NB, V], dt)
            z = misc.tile([NB, V], dt)
            z_bf = misc.tile([NB, V], BF16)
            zT = misc.tile([128, 4, NB], BF16)
            out_sb = misc.tile([NB, V], dt)

            def load_mlp_weights():
                nc.scalar.dma_start(out=wg1T[:], in_=wg1T_t[:].rearrange("(c p) j -> p c j", p=128))
                nc.scalar.dma_start(out=wg2T[:], in_=wg2T_t[:].rearrange("(c p) j -> p c j", p=128))
                nc.scalar.dma_start(out=woT[:], in_=woT_t[:].rearrange("(c p) j -> p c j", p=128))
                nc.scalar.dma_start(out=bg1_row[:], in_=bg1_t[None, :])
                nc.scalar.dma_start(out=bg2_row[:], in_=bg2_t[None, :])
                nc.scalar.dma_start(out=bo_row[:], in_=bo_t[None, :])

            def exps_half(h0, H):
                """write-row correction exps for batches [h0, h0+H)."""
                hs = slice(h0, h0 + H)
                nc.scalar.activation(out=eo[hs], in_=sold[hs],
                                     func=mybir.ActivationFunctionType.Exp,
                                     bias=neg_m_all[hs, :1], scale=1.0 / SCALE)
                nc.scalar.activation(out=en[hs], in_=snew[hs],
                                     func=mybir.ActivationFunctionType.Exp,
                                     bias=neg_m_all[hs, :1], scale=1.0 / SCALE)

            def corr_half(h0, H):
                """denominator + retrieved for batches [h0, h0+H) (DVE)."""
                hs = slice(h0, h0 + H)
                nc.vector.tensor_tensor(out=mask_wp[hs], in0=wp_sb[hs],
                                        in1=filled_sb[hs], op=mybir.AluOpType.is_lt)
                nc.vector.tensor_mul(out=a_old[hs], in0=eo[hs], in1=mask_wp[hs])
                nc.vector.tensor_mul(out=a_new[hs], in0=en[hs], in1=mask_wp[hs])
                nc.vector.tensor_sub(out=denom[hs], in0=denom0[hs], in1=a_old[hs])
                nc.vector.tensor_add(out=denom[hs], in0=denom[hs], in1=a_new[hs])
                nc.vector.reciprocal(out=recip[hs], in_=denom[hs])

                nc.vector.tensor_scalar_mul(out=t1[hs], in0=value_sb[hs],
                                            scalar1=a_new[hs, :1])
                nc.vector.tensor_scalar_mul(out=t2_[hs], in0=vwp_sb[hs],
                                            scalar1=a_old[hs, :1])
                nc.vector.tensor_sub(out=t1[hs], in0=t1[hs], in1=t2_[hs])
                nc.vector.tensor_add(out=t1[hs], in0=g_sb[hs], in1=t1[hs])
                nc.vector.tensor_scalar_mul(out=retr[hs], in0=t1[hs],
                                            scalar1=recip[hs, :1])
                nc.vector.tensor_copy(out=retr_bf[hs], in_=retr[hs])

            def mlp_half(h0, H):
                """gated MLP for batches [h0, h0+H) (PE-heavy)."""
                hs = slice(h0, h0 + H)
                idb = identity_bf[hs, hs]   # 32x32 identity block at base h0
                for c in range(4):
                    tp = ps_tr.tile([128, H], BF16, tag="tr")
                    nc.tensor.transpose(out=tp[:], in_=retr_bf[hs, c * 128:(c + 1) * 128],
                                        identity=idb)
                    nc.scalar.copy(out=rT[:, c, h0:h0 + H], in_=tp[:])

                g_ps = ps_tr.tile([NB, V], dt, tag="tr")
                for ic in range(8):
                    lhsT = hT[:, ic, h0:h0 + H] if ic < 4 else rT[:, ic - 4, h0:h0 + H]
                    nc.tensor.matmul(out=g_ps[hs, :], lhsT=lhsT, rhs=wg1T[:, ic, :],
                                     start=(ic == 0), stop=False)
                nc.tensor.matmul(out=g_ps[hs, :], lhsT=ones_bf[:, :H], rhs=bg1_row[:],
                                 start=False, stop=True)
                nc.scalar.activation(out=g_act[hs], in_=g_ps[hs, :],
                                     func=mybir.ActivationFunctionType.Sigmoid)
                nc.vector.tensor_mul(out=g_act[hs], in0=g_act[hs], in1=g_ps[hs, :])

                nc.vector.tensor_copy(out=g_act_bf[hs], in_=g_act[hs])
                for c in range(4):
                    tp = ps_tr.tile([128, H], BF16, tag="tr")
                    nc.tensor.transpose(out=tp[:], in_=g_act_bf[hs, c * 128:(c + 1) * 128],
                                        identity=idb)
                    nc.scalar.copy(out=gT[:, c, h0:h0 + H], in_=tp[:])

                gate_ps = ps_tr.tile([NB, V], dt, tag="tr")
                for c in range(4):
                    nc.tensor.matmul(out=gate_ps[hs, :], lhsT=gT[:, c, h0:h0 + H],
                                     rhs=wg2T[:, c, :], start=(c == 0), stop=False)
                nc.tensor.matmul(out=gate_ps[hs, :], lhsT=ones_bf[:, :H],
                                 rhs=bg2_row[:], start=False, stop=True)
                nc.scalar.activation(out=gate[hs], in_=gate_ps[hs, :],
                                     func=mybir.ActivationFunctionType.Sigmoid)

                nc.vector.tensor_mul(out=z[hs], in0=gate[hs], in1=retr[hs])
                nc.vector.tensor_add(out=z[hs], in0=z[hs], in1=hidden_sb[hs])
                nc.vector.tensor_copy(out=z_bf[hs], in_=z[hs])
                for c in range(4):
                    tp = ps_tr.tile([128, H], BF16, tag="tr")
                    nc.tensor.transpose(out=tp[:], in_=z_bf[hs, c * 128:(c + 1) * 128],
                                        identity=idb)
                    nc.scalar.copy(out=zT[:, c, h0:h0 + H], in_=tp[:])

                out_ps = ps_tr.tile([NB, V], dt, tag="tr")
                for c in range(4):
                    nc.tensor.matmul(out=out_ps[hs, :], lhsT=zT[:, c, h0:h0 + H],
                                     rhs=woT[:, c, :], start=(c == 0), stop=False)
                nc.tensor.matmul(out=out_ps[hs, :], lhsT=ones_bf[:, :H],
                                 rhs=bo_row[:], start=False, stop=True)
                nc.vector.tensor_copy(out=out_sb[hs], in_=out_ps[hs, :])
                nc.sync.dma_start(out=out_t[hs, :], in_=out_sb[hs])

            H2 = NB // 2
            scores_stage(0)
            values_stage(0)
            load_mlp_weights()
            scores_stage(1)
            values_stage(1)
            exps_half(0, H2)    # groups 0-1 done: first-half corrections
            corr_half(0, H2)    # overlap the DMA-bound middle (DVE/scalar)
            scores_stage(2)
            values_stage(2)
            scores_stage(3)
            exps_half(H2, H2)   # needs neg_m of groups 2-3 only
            values_stage(3)
            corr_half(H2, H2)
            mlp_half(0, NB)     # single full-width MLP (split was a net loss)

    nc.finalize()
    return nc


_NC_CACHE = {}


def _get_nc(pv2, limg):
    key = (tuple(pv2), tuple(limg))
    if key not in _NC_CACHE:
        _NC_CACHE.clear()
        _NC_CACHE[key] = _build(tuple(pv2), tuple(limg))
    return _NC_CACHE[key]


def _make_plan(filled):
    fl = np.asarray(filled).astype(np.int64)
    f_w = np.minimum(fl + 1, S)
    order = np.argsort(-f_w, kind="stable")
    idx = order.reshape(NB, NCORES)          # slot i, core c -> batch idx[i, c]
    fmax = f_w[idx[:, 0]]
    pv2 = np.minimum((fmax + 255) // 256, T2).astype(np.int64)
    limg = tuple(int(min((fmax[g * GRP] + 7) // 8 * 8, S)) for g in range(NG))
    return idx, tuple(int(x) for x in pv2), limg


def _make_in_maps(idx, keys, values, key, value, hidden, write_ptr, filled,
                  Wq, bq, Wg1, bg1, Wg2, bg2, Wo, bo):
    f32 = np.float32
    bidx = np.arange(B)
    wp = np.asarray(write_ptr).astype(np.int64)
    fl = np.asarray(filled).astype(np.int64)

    keys_f8 = np.asarray(keys, dtype=f32).astype(NP_F8)
    kwp = keys_f8[bidx, wp].astype(f32)
    keysT = np.ascontiguousarray(keys_f8.transpose(0, 2, 1))      # [B, K, S]

    values_f8 = np.asarray(values, dtype=f32).astype(NP_F8)
    vwp = values_f8[bidx, wp].astype(f32)
    # vpack[b, p, t2, i, v] = values[b, t2*256 + i*128 + p, v]
    vpack = np.ascontiguousarray(
        values_f8.reshape(B, T2, 2, 128, V).transpose(0, 3, 1, 2, 4))

    key = np.asarray(key, dtype=f32)
    value = np.asarray(value, dtype=f32)
    hidden = np.asarray(hidden, dtype=f32)

    wqT = np.ascontiguousarray(np.asarray(Wq, dtype=f32).T).astype(NP_BF16)
    wg1T = np.ascontiguousarray(np.asarray(Wg1, dtype=f32).T).astype(NP_BF16)
    wg2T = np.ascontiguousarray(np.asarray(Wg2, dtype=f32).T).astype(NP_BF16)
    woT = np.ascontiguousarray(np.asarray(Wo, dtype=f32).T).astype(NP_BF16)
    bq = np.ascontiguousarray(np.asarray(bq, dtype=f32)).astype(NP_BF16)
    bg1 = np.ascontiguousarray(np.asarray(bg1, dtype=f32)).astype(NP_BF16)
    bg2 = np.ascontiguousarray(np.asarray(bg2, dtype=f32)).astype(NP_BF16)
    bo = np.ascontiguousarray(np.asarray(bo, dtype=f32)).astype(NP_BF16)

    filled_w = np.minimum(fl + 1, S).astype(f32).reshape(B, 1)
    wp_f = wp.astype(f32).reshape(B, 1)

    in_maps = []
    for c in range(NCORES):
        sel = idx[:, c]
        in_maps.append({
            "keysT": np.ascontiguousarray(keysT[sel]),
            "vpack": np.ascontiguousarray(vpack[sel]),
            "key": np.ascontiguousarray(key[sel]),
            "value": np.ascontiguousarray(value[sel]),
            "hidden": np.ascontiguousarray(hidden[sel]),
            "filled_f": np.ascontiguousarray(filled_w[sel]),
            "wp_f": np.ascontiguousarray(wp_f[sel]),
            "kwp": np.ascontiguousarray(kwp[sel]),
            "vwp": np.ascontiguousarray(vwp[sel]),
            "WqT": wqT, "Wg1T": wg1T, "Wg2T": wg2T, "WoT": woT,
            "bq": bq, "bg1": bg1, "bg2": bg2, "bo": bo,
        })
    return in_maps


def run(trace=False, **inputs):
    idx, pv2, limg = _make_plan(inputs["filled"])
    nc = _get_nc(pv2, limg)
    in_maps = _make_in_maps(idx, **inputs)
    res = run_bass_kernel_spmd(nc, in_maps, core_ids=list(range(NCORES)),
                               trace=trace)
    out = np.empty((B, V), np.float32)
    for c in range(NCORES):
        out[idx[:, c]] = res.results[c]["out"]
    return out, res


def kernel(**inputs) -> np.ndarray:
    out, _ = run(trace=False, **inputs)
    return out
